# revision 5
# baseline (speedup 1.0000x reference)
"""Trainium2 Bass kernel for nn_AttentionBlock (RMSNorm + QKV + causal
attention with softmax over the QUERY axis + output projection).

Sharding: data-parallel over batch. B=8 -> one batch element per NeuronCore,
no collectives. Weights are re-laid-out on the host (de-interleave the
(h, dh, 3) QKV packing, transpose so the contraction dim d lands on SBUF
partitions, and pre-tile so every DMA is contiguous).

Device dataflow per core (S=1024, D=1024, H=16, Dh=64):
  1. x natural [s,d] tiles (bf16) -> sum(x^2) via DVE multiply+accum ->
     rsqrt scale -> xbf (bf16) -> PE-transpose (batched 4 per PSUM bank)
     -> normT [d, s] in one unified SBUF tile.
  2. qkT[f, s] = Wqk^T.T @ normT (a 128-row f-tile holds a HEAD PAIR);
     V[s, f] = normT.T @ Wv^T.  Projection loops run the weight tile
     (stationary operand) in the OUTER loop over both 512-wide chunks so
     each LDWEIGHTS serves two matmuls; each projection tile drains its
     2-bank PSUM tile with a single DVE cast.
  3. scores per (pair, k-tile) land in 2-bank PSUM tiles so the full
     valid q-row [ki*128, 1024) is contiguous; causal masking is a DVE
     add of a -1e30 triangle on the diagonal block; ONE Exp per
     (head, k-tile) with accum_out giving the full row sum for free.
  4. normalization folded into V (vsc = V * 1/rowsum);
     z^T[dh, q] = vsc.T @ attnT, two heads into one PSUM bank via
     column-group tiling.
  5. out[s, :] = z.T @ Wo^T accumulated over f tiles, DMA out per chunk.

All matmul operands are bfloat16.  The pair loop is software-pipelined:
the next pair's QKV projections are emitted between the scores and the
final z chunk so the PE always has independent work while ACT drains exps.
"""

import numpy as np
from contextlib import ExitStack

import concourse.bacc as bacc
import concourse.bass as bass
import concourse.tile as tile
from concourse import mybir
from concourse.bass_utils import run_bass_kernel_spmd

B, S, DM, H, DH = 8, 1024, 1024, 16, 64
P = 128
EPS = 1.1920929e-07
NEG = -1e30
F32 = mybir.dt.float32
BF16 = mybir.dt.bfloat16
NS = S // P      # 8 s-tiles (also k-tiles)
ND = DM // P     # 8 d-chunks
NPAIR = H // 2   # 8 head pairs
QCH = 512        # one PSUM bank of fp32
NQC = S // QCH   # 2 q chunks

MMDT = BF16
PDT = BF16


def build_program(with_bias=False):
    nc = bacc.Bacc("TRN2", target_bir_lowering=False, debug=False)

    xb = nc.dram_tensor("xb", [S, DM], PDT, kind="ExternalInput").ap()
    # (ft, dd, dk, f): ft 0-7 = Q pair tiles, 8-15 = K pair tiles
    wqk = nc.dram_tensor("wqk", [2 * NPAIR, P, ND, P], PDT, kind="ExternalInput").ap()
    wv = nc.dram_tensor("wv", [ND, P, DM], PDT, kind="ExternalInput").ap()
    wo = nc.dram_tensor("wo", [ND, P, DM], PDT, kind="ExternalInput").ap()
    ident = nc.dram_tensor("ident", [P, P], PDT, kind="ExternalInput").ap()
    trimask = nc.dram_tensor("trimask", [P, P], F32, kind="ExternalInput").ap()
    bqk = bv = None
    if with_bias:
        bqk = nc.dram_tensor("bqk", [P, 2 * NPAIR], F32, kind="ExternalInput").ap()
        bv = nc.dram_tensor("bv", [1, DM], PDT, kind="ExternalInput").ap()
    out = nc.dram_tensor("out", [S, DM], F32, kind="ExternalOutput").ap()

    with tile.TileContext(nc) as tc:
        with ExitStack() as ctx:
            _build_body(ctx, tc, xb, wqk, wv, wo, ident, trimask, bqk, bv, out)
    nc.compile()
    return nc


def _build_body(ctx, tc, xb, wqk, wv, wo, ident, trimask, bqk, bv, out):
    nc = tc.nc
    AF = mybir.ActivationFunctionType
    ALU = mybir.AluOpType

    singles = ctx.enter_context(tc.tile_pool(name="singles", bufs=1))
    big8 = ctx.enter_context(tc.tile_pool(name="big8", bufs=1))    # V tiles
    xqp = ctx.enter_context(tc.tile_pool(name="xqp", bufs=1))      # x natural
    xbfp = ctx.enter_context(tc.tile_pool(name="xbfp", bufs=2))    # scaled bf16 x
    scr = ctx.enter_context(tc.tile_pool(name="scr", bufs=2))      # out tiles
    nt = ctx.enter_context(tc.tile_pool(name="nt", bufs=1))        # normT
    w8 = ctx.enter_context(tc.tile_pool(name="w8", bufs=1))        # wv then wo
    wqks = ctx.enter_context(tc.tile_pool(name="wqks", bufs=1))    # wqk stream
    qkp = ctx.enter_context(tc.tile_pool(name="qkp", bufs=2))      # qt/kt tiles
    attnp = ctx.enter_context(tc.tile_pool(name="attnp", bufs=2))  # exp(scores)
    ztp = ctx.enter_context(tc.tile_pool(name="ztp", bufs=1))      # z transposed
    vscp = ctx.enter_context(tc.tile_pool(name="vscp", bufs=2))    # scaled V
    sm = ctx.enter_context(tc.tile_pool(name="sm", bufs=3))        # small stats
    ps = ctx.enter_context(tc.tile_pool(name="ps", bufs=1, space="PSUM"))

    # PSUM budget (8 banks): tag "sc" = [P, 1024] f32 (2 banks) x bufs 3,
    # tag "z" = [P, 512] f32 (1 bank) x bufs 2.
    def sc_tile(name):
        return ps.tile([P, 2 * QCH], F32, tag="sc", bufs=3, name=name)

    def z_bank(name):
        return ps.tile([P, QCH], F32, tag="z", bufs=2, name=name)

    # ---------------- Phase A: RMSNorm + transpose ----------------
    # x tile 0 heads the RMS->transpose critical path, so its DMA goes
    # first; ident (needed by the first transpose) right after.
    x_qs = []
    for qi in range(NS // 2):
        x_q = xqp.tile([P, 2, DM], PDT, tag=f"xq{qi}", name=f"xq{qi}")
        x_qs.append(x_q)
    nc.sync.dma_start(out=x_qs[0][:, 0, :], in_=xb[0:P, :])
    ident_sb = singles.tile([P, P], PDT, tag="ident")
    nc.sync.dma_start(out=ident_sb, in_=ident)
    nc.sync.dma_start(out=x_qs[0][:, 1, :], in_=xb[P:2 * P, :])
    for qi in range(1, NS // 2):
        nc.sync.dma_start(
            out=x_qs[qi],
            in_=xb[qi * 2 * P:(qi + 1) * 2 * P, :].rearrange(
                "(j p) d -> p j d", j=2))
    eps_sb = singles.tile([P, 1], F32, tag="eps")
    nc.vector.memset(eps_sb, EPS)
    # touch Sqrt and Exp now so their ACT table loads (~1.3us each) happen
    # during the input DMA instead of on the first real use
    warm_sb = singles.tile([P, 1], F32, tag="warm")
    nc.scalar.activation(out=warm_sb, in_=eps_sb, func=AF.Sqrt)
    nc.scalar.activation(out=warm_sb, in_=eps_sb, func=AF.Exp)
    tri_sb = singles.tile([P, P], F32, tag="tri")
    nc.sync.dma_start(out=tri_sb, in_=trimask)
    wqk_pre = {}
    for ft in (0, NPAIR, 1, NPAIR + 1):
        w_t = wqks.tile([P, ND, P], PDT, tag="wqk", bufs=4, name=f"wqk{ft}")
        nc.sync.dma_start(out=w_t, in_=wqk[ft])
        wqk_pre[ft] = w_t
    bqk_sb = bv_sb = ones_sb = None
    if bqk is not None:
        bqk_sb = singles.tile([P, 2 * NPAIR], F32, tag="bqk")
        nc.sync.dma_start(out=bqk_sb, in_=bqk)
        bv_sb = singles.tile([1, DM], PDT, tag="bv")
        nc.sync.dma_start(out=bv_sb, in_=bv)
        ones_sb = singles.tile([1, P], PDT, tag="ones")
        nc.vector.memset(ones_sb, 1.0)

    # unified normT: nt_all[:, dk, s]
    nt_all = nt.tile([P, ND, S], PDT, tag="nt", name="normT")

    # ---------------- QK projection helper ----------------
    qt_tiles = {}
    kt_tiles = {}

    def dma_wqk(t):
        for ft in (t, NPAIR + t):
            if ft not in wqk_pre:
                w_t = wqks.tile([P, ND, P], PDT, tag="wqk", bufs=4,
                                name=f"wqk{ft}")
                nc.sync.dma_start(out=w_t, in_=wqk[ft])
                wqk_pre[ft] = w_t

    def emit_qk_chunk(t, sc):
        """qkT s-chunk sc for pair t: Q (ft=t) and K (ft=8+t).  The dk
        (weight) loop is OUTER so each stationary tile serves both chunks
        when emitted via emit_qk_pair; chunk-split emission still works."""
        for which, ft in (("qt", t), ("kt", NPAIR + t)):
            tiles = qt_tiles if which == "qt" else kt_tiles
            if sc == 0:
                w_t = wqk_pre.pop(ft)
                dst = qkp.tile([P, S], PDT, tag=which, name=f"{which}{t}")
                mm = sc_tile(f"qkps{ft}")
                tiles[t] = (dst, w_t, mm)
            dst, w_t, mm = tiles[t]
            for dk in range(ND):
                nc.tensor.matmul(
                    mm[:, sc * QCH:(sc + 1) * QCH],
                    w_t[:, dk, :],
                    nt_all[:, dk, sc * QCH:(sc + 1) * QCH],
                    start=(dk == 0), stop=(dk == ND - 1),
                )
            if sc == 1:
                dview = dst.rearrange("p (c q) -> p c q", c=2)
                mview = mm.rearrange("p (c q) -> p c q", c=2)
                if bqk_sb is not None:
                    nc.vector.tensor_scalar_add(out=dview, in0=mview,
                                                scalar1=bqk_sb[:, ft:ft + 1])
                else:
                    nc.vector.tensor_copy(out=dview, in_=mview)

    def emit_qk_pair(t):
        """Both chunks with the dk loop outermost: one LDWEIGHTS per
        (which, dk) covers two 512-wide matmuls."""
        for which, ft in (("qt", t), ("kt", NPAIR + t)):
            tiles = qt_tiles if which == "qt" else kt_tiles
            w_t = wqk_pre.pop(ft)
            dst = qkp.tile([P, S], PDT, tag=which, name=f"{which}{t}")
            mm = sc_tile(f"qkps{ft}")
            tiles[t] = (dst, w_t, mm)
            for dk in range(ND):
                for sc in range(NQC):
                    nc.tensor.matmul(
                        mm[:, sc * QCH:(sc + 1) * QCH],
                        w_t[:, dk, :],
                        nt_all[:, dk, sc * QCH:(sc + 1) * QCH],
                        start=(dk == 0), stop=(dk == ND - 1),
                    )
            dview = dst.rearrange("p (c q) -> p c q", c=2)
            mview = mm.rearrange("p (c q) -> p c q", c=2)
            if bqk_sb is not None:
                nc.vector.tensor_scalar_add(out=dview, in0=mview,
                                            scalar1=bqk_sb[:, ft:ft + 1])
            else:
                nc.vector.tensor_copy(out=dview, in_=mview)

    for st in range(NS):
        x_t = x_qs[st // 2][:, st % 2, :]
        sqscr = sm.tile([P, DM], PDT, tag="sqscr", bufs=1, name=f"sqscr{st}")
        ssum = sm.tile([P, 1], F32, tag="ssA", name=f"ssA{st}")
        nc.vector.scalar_tensor_tensor(
            out=sqscr, in0=x_t, scalar=1.0, in1=x_t,
            op0=ALU.mult, op1=ALU.mult, accum_out=ssum)
        rs_t = sm.tile([P, 1], F32, tag="rs", name=f"rs{st}")
        # rs = sqrt(mean(x^2) + eps) then reciprocal
        nc.scalar.activation(out=rs_t, in_=ssum, func=AF.Sqrt,
                             bias=eps_sb, scale=1.0 / DM)
        nc.vector.reciprocal(out=rs_t, in_=rs_t)
        xbf = xbfp.tile([P, DM], PDT, tag="xbf", name=f"xbf{st}")
        nc.vector.tensor_scalar_mul(out=xbf, in0=x_t, scalar1=rs_t)
        # batched PE transposes: 4 per PSUM bank, one wide cast to normT
        for g in range(2):
            tp_ps = ps.tile([P, 4, P], PDT, tag="z", bufs=2, name=f"tp{st}_{g}")
            for j in range(4):
                dk = 4 * g + j
                nc.tensor.transpose(tp_ps[:, j, 0:P],
                                    xbf[:, dk * P:(dk + 1) * P], ident_sb)
            nc.scalar.copy(
                out=nt_all[:, 4 * g:4 * g + 4, st * P:(st + 1) * P],
                in_=tp_ps[:, :, 0:P])
        # normT cols 0..511 complete: give the PE pair-0 projection work so
        # it isn't stuck behind transposes that wait on the scalar RMS chain
        if st == 3:
            emit_qk_chunk(0, 0)

    # ---------------- Phase B: rest of QK pairs 0,1 then V ----------------
    emit_qk_chunk(0, 1)
    emit_qk_pair(1)

    wv_sb = []
    for dk in range(ND):
        w_t = w8.tile([P, DM], PDT, tag=f"w{dk}", name=f"wv{dk}")
        nc.sync.dma_start(out=w_t, in_=wv[dk])
        wv_sb.append(w_t)
    vs = []
    for st in range(NS):
        v_t = big8.tile([P, DM], PDT, tag=f"b{st}", name=f"vs{st}")
        vs.append(v_t)
        mm = sc_tile(f"vps{st}")
        for dk in range(ND):
            for fvc in range(NQC):
                nc.tensor.matmul(
                    mm[:, fvc * QCH:(fvc + 1) * QCH],
                    nt_all[:, dk, st * P:(st + 1) * P],
                    wv_sb[dk][:, fvc * QCH:(fvc + 1) * QCH],
                    start=(dk == 0),
                    stop=(dk == ND - 1 and bv_sb is None),
                )
        if bv_sb is not None:
            for fvc in range(NQC):
                nc.tensor.matmul(
                    mm[:, fvc * QCH:(fvc + 1) * QCH], ones_sb,
                    bv_sb[:, fvc * QCH:(fvc + 1) * QCH],
                    start=False, stop=True,
                )
        nc.vector.tensor_copy(
            out=v_t.rearrange("p (c q) -> p c q", c=2),
            in_=mm.rearrange("p (c q) -> p c q", c=2))

    # prefetch Wo early: the w8 slots free up as soon as V consumed wv
    wo_sb = []
    for fk in range(ND):
        w_t = w8.tile([P, DM], PDT, tag=f"w{fk}", name=f"wo{fk}")
        nc.sync.dma_start(out=w_t, in_=wo[fk])
        wo_sb.append(w_t)

    # ---------------- Phase E emitter (out projection per s-tile) --------
    zT = []

    def emit_out_tile(st):
        o_t = scr.tile([P, DM], F32, tag="osb", name=f"osb{st}")
        if st == NS - 1:
            # quarter-width groups on z banks: the copy+DMA chain overlaps
            # the remaining matmuls instead of serializing at kernel end
            w = QCH // 2
            for half in range(2):
                mm = z_bank(f"ops{st}_{half}")
                for sub in range(2):
                    dmc = 2 * half + sub
                    for fk in range(ND):
                        nc.tensor.matmul(
                            mm[:, sub * w:(sub + 1) * w],
                            zT[fk][:, st * P:(st + 1) * P],
                            wo_sb[fk][:, dmc * w:(dmc + 1) * w],
                            start=(fk == 0), stop=(fk == ND - 1),
                        )
                    nc.vector.tensor_copy(
                        out=o_t[:, dmc * w:(dmc + 1) * w],
                        in_=mm[:, sub * w:(sub + 1) * w])
                    nc.sync.dma_start(
                        out=out[st * P:(st + 1) * P, dmc * w:(dmc + 1) * w],
                        in_=o_t[:, dmc * w:(dmc + 1) * w])
            return
        mm = sc_tile(f"ops{st}")
        for fk in range(ND):
            for dmc in range(NQC):
                nc.tensor.matmul(
                    mm[:, dmc * QCH:(dmc + 1) * QCH],
                    zT[fk][:, st * P:(st + 1) * P],
                    wo_sb[fk][:, dmc * QCH:(dmc + 1) * QCH],
                    start=(fk == 0), stop=(fk == ND - 1),
                )
        for dmc in range(NQC):
            nc.vector.tensor_copy(out=o_t[:, dmc * QCH:(dmc + 1) * QCH],
                                  in_=mm[:, dmc * QCH:(dmc + 1) * QCH])
            nc.sync.dma_start(
                out=out[st * P:(st + 1) * P, dmc * QCH:(dmc + 1) * QCH],
                in_=o_t[:, dmc * QCH:(dmc + 1) * QCH])

    # ---------------- Phase C/D: attention per head pair ----------------
    for t in range(NPAIR):
        if t + 2 < NPAIR:
            dma_wqk(t + 2)
        qt, kt = qt_tiles.pop(t)[0], kt_tiles.pop(t)[0]

        z_t = ztp.tile([P, S], PDT, tag=f"zt{t}", name=f"zT{t}")
        zT.append(z_t)
        attn = {}   # (head_local, ki) -> sbuf tile [P, width]
        vsc_d = {}  # ki -> scaled V slice [P, 128] for this pair
        rsp_d = {}  # ki -> row sums [P, 2] (hl 0, 1)

        def emit_ri_vsc(ki, t=t, vsc_d=vsc_d, rsp_d=rsp_d):
            ri = sm.tile([P, 2], F32, tag="ri", name=f"ri{t}_{ki}")
            nc.vector.reciprocal(out=ri, in_=rsp_d[ki])
            vsc = vscp.tile([P, P], MMDT, tag=f"vsc{ki}", name=f"vsc{t}_{ki}")
            vsc_d[ki] = vsc
            ri_b = bass.AP(tensor=ri.tensor, offset=ri.offset,
                           ap=[list(ri.ap[0]), list(ri.ap[1]), [0, DH]])
            nc.vector.tensor_tensor(
                out=vsc.rearrange("p (h d) -> p h d", h=2),
                in0=vs[ki][:, t * P:(t + 1) * P].rearrange(
                    "p (h d) -> p h d", h=2),
                in1=ri_b, op=ALU.mult)

        z_ps = {}
        for qc in range(NQC):
            z_ps[qc] = z_bank(f"zps{t}_{qc}")

        def emit_z_contrib(ki, last1=False, t=t, attn=attn, vsc_d=vsc_d,
                           z_ps=z_ps):
            chunks = ([(0, ki == 3)] if ki < 4 else []) + [(1, last1)]
            for qc, stop in chunks:
                q0 = max(qc * QCH, ki * P)
                for hl in (0, 1):
                    nc.tensor.matmul(
                        z_ps[qc][hl * DH:(hl + 1) * DH, q0 - qc * QCH:QCH],
                        vsc_d[ki][:, hl * DH:(hl + 1) * DH],
                        attn[(hl, ki)][:, q0 - ki * P:(qc + 1) * QCH - ki * P],
                        start=(ki == 0), stop=stop,
                    )

        def finish_z_chunk(qc, t=t, z_t=z_t, z_ps=z_ps):
            # chunk 0's cast goes to scalar (emitted after the pair's last
            # exp, so it can't delay them); chunk 1's to vector
            if qc == 0 and t != NPAIR - 1:
                nc.scalar.copy(out=z_t[:, qc * QCH:(qc + 1) * QCH],
                               in_=z_ps[qc])
            else:
                nc.vector.tensor_copy(
                    out=z_t[:, qc * QCH:(qc + 1) * QCH], in_=z_ps[qc])

        for ki in range(NS):
            width = S - ki * P
            rsp = sm.tile([P, 2], F32, tag="rsp", name=f"rsp{t}_{ki}")
            rsp_d[ki] = rsp
            if ki < 4:
                # two 2-bank tiles, one per head; each holds the full
                # valid q span [ki*128, 1024) contiguously
                s_hl = [sc_tile(f"sps{t}_{hl}_{ki}") for hl in (0, 1)]
                for qc in range(NQC):
                    q0 = max(qc * QCH, ki * P)
                    for hl, prange in ((0, slice(0, DH)), (1, slice(DH, P))):
                        nc.tensor.matmul(
                            s_hl[hl][:, q0:(qc + 1) * QCH],
                            kt[prange, ki * P:(ki + 1) * P],
                            qt[prange, q0:(qc + 1) * QCH],
                            start=True, stop=True,
                        )
                for hl in (0, 1):
                    nc.vector.tensor_add(
                        out=s_hl[hl][:, ki * P:(ki + 1) * P],
                        in0=s_hl[hl][:, ki * P:(ki + 1) * P], in1=tri_sb)
                for hl in (0, 1):
                    a_t = attnp.tile([P, width], MMDT, tag=f"at{ki}",
                                     name=f"attn{t}_{hl}_{ki}")
                    attn[(hl, ki)] = a_t
                    nc.scalar.activation(
                        out=a_t, in_=s_hl[hl][:, ki * P:2 * QCH],
                        func=AF.Exp, accum_out=rsp[:, hl:hl + 1])
            else:
                # one 2-bank tile: bank 0 = head 0, bank 1 = head 1
                s01 = sc_tile(f"sps{t}_{ki}")
                off = ki * P - QCH
                for hl, prange in ((0, slice(0, DH)), (1, slice(DH, P))):
                    nc.tensor.matmul(
                        s01[:, hl * QCH + off:(hl + 1) * QCH],
                        kt[prange, ki * P:(ki + 1) * P],
                        qt[prange, ki * P:S],
                        start=True, stop=True,
                    )
                for hl in (0, 1):
                    nc.vector.tensor_add(
                        out=s01[:, hl * QCH + off:hl * QCH + off + P],
                        in0=s01[:, hl * QCH + off:hl * QCH + off + P],
                        in1=tri_sb)
                for hl in (0, 1):
                    a_t = attnp.tile([P, width], MMDT, tag=f"at{ki}",
                                     name=f"attn{t}_{hl}_{ki}")
                    attn[(hl, ki)] = a_t
                    nc.scalar.activation(
                        out=a_t, in_=s01[:, hl * QCH + off:(hl + 1) * QCH],
                        func=AF.Exp, accum_out=rsp[:, hl:hl + 1])
            # recip+vsc for the PREVIOUS ki (keeps the in-order vector
            # queue from gating the next exp)
            if ki >= 1:
                emit_ri_vsc(ki - 1)
            if ki >= 3:
                emit_z_contrib(ki - 3)
        finish_z_chunk(0)
        # cover the wait for exp(5..7) with independent matmuls: the next
        # pair's projection (or the first out tiles on the last pair)
        if t + 2 < NPAIR:
            emit_qk_pair(t + 2)
        if t == NPAIR - 1:
            for st in range(4):
                emit_out_tile(st)
            emit_ri_vsc(NS - 1)
            emit_z_contrib(NS - 3)
            emit_z_contrib(NS - 2)
            emit_z_contrib(NS - 1, last1=True)
            finish_z_chunk(1)
            for st in range(4, NS):
                emit_out_tile(st)
        else:
            emit_ri_vsc(NS - 1)
            emit_z_contrib(NS - 3)
            emit_z_contrib(NS - 2)
            emit_z_contrib(NS - 1, last1=True)
            finish_z_chunk(1)


def prep_inputs(x, W_qkv, b_qkv):
    """Host-side re-layout of inputs (weights de-interleave/transpose/tile)."""
    x = np.ascontiguousarray(np.asarray(x, np.float32)).astype(NP_PDT)
    W = np.asarray(W_qkv, np.float32).reshape(H, DH, 3, DM)
    Wq = W[:, :, 0, :].reshape(H * DH, DM)
    Wk = W[:, :, 1, :].reshape(H * DH, DM)
    Wv = W[:, :, 2, :].reshape(H * DH, DM)
    WqkT = np.ascontiguousarray(np.concatenate([Wq, Wk], 0).T)   # [DM, 2048]
    wqk_host = np.ascontiguousarray(
        WqkT.reshape(ND, P, 2 * NPAIR, P).transpose(2, 1, 0, 3)).astype(NP_PDT)
    wv_host = np.ascontiguousarray(Wv.T).reshape(ND, P, DM).astype(NP_PDT)
    ident = np.eye(P, dtype=np.float32).astype(NP_PDT)
    idx = np.arange(P)
    trimask = np.where(idx[None, :] >= idx[:, None], 0.0, NEG).astype(np.float32)

    b = np.asarray(b_qkv, np.float32).reshape(H, DH, 3)
    bq = b[:, :, 0].reshape(H * DH)
    bk = b[:, :, 1].reshape(H * DH)
    bvv = b[:, :, 2].reshape(H * DH)
    bqk_host = np.ascontiguousarray(
        np.concatenate([bq, bk]).reshape(2 * NPAIR, P).T)         # [P, 16]
    return x, wqk_host, wv_host, ident, trimask, bqk_host, bvv


import ml_dtypes

NP_PDT = ml_dtypes.bfloat16

_prog_cache = {}


def kernel(x, W_qkv, b_qkv, W_o, b_o, trace=False):
    x, wqk_host, wv_host, ident, trimask, bqk_host, bvv = prep_inputs(
        x, W_qkv, b_qkv)
    wo_host = np.ascontiguousarray(np.asarray(W_o, np.float32).T).reshape(ND, P, DM).astype(NP_PDT)
    with_bias = bool(np.any(np.asarray(b_qkv)))
    key = with_bias
    if key not in _prog_cache:
        _prog_cache[key] = build_program(with_bias=with_bias)
    nc = _prog_cache[key]

    in_maps = []
    for bi in range(B):
        m = {
            "xb": x[bi], "wqk": wqk_host, "wv": wv_host, "wo": wo_host,
            "ident": ident, "trimask": trimask,
        }
        if with_bias:
            m["bqk"] = bqk_host
            m["bv"] = bvv.reshape(1, DM).astype(NP_PDT)
        in_maps.append(m)

    res = run_bass_kernel_spmd(nc, in_maps, core_ids=list(range(B)), trace=trace)
    out = np.stack([res.results[bi]["out"] for bi in range(B)]).astype(np.float32)
    out += np.asarray(b_o, np.float32)[None, None, :]
    if trace:
        kernel.last_results = res
    return out


# revision 12
# speedup vs baseline: 1.1683x; 1.1683x over previous
"""Trainium2 Bass kernel for nn_AttentionBlock (RMSNorm + QKV + causal
attention with softmax over the QUERY axis + output projection).

Sharding: data-parallel over batch. B=8 -> one batch element per NeuronCore,
no collectives. Weights are re-laid-out on the host (de-interleave the
(h, dh, 3) QKV packing, transpose so the contraction dim d lands on SBUF
partitions, and pre-tile so every DMA is contiguous).

Device dataflow per core (S=1024, D=1024, H=16, Dh=64):
  1. x natural [s,d] tiles (bf16) -> sum(x^2) via DVE multiply+accum ->
     rsqrt scale -> xbf (bf16) -> PE-transpose (batched 4 per PSUM bank)
     -> normT [d, s] in one unified SBUF tile.
  2. qkT[f, s] = Wqk^T.T @ normT (a 128-row f-tile holds a HEAD PAIR);
     V[s, f] = normT.T @ Wv^T.  Projection loops run the weight tile
     (stationary operand) in the OUTER loop over both 512-wide chunks so
     each LDWEIGHTS serves two matmuls; each projection tile drains its
     2-bank PSUM tile with a single DVE cast.
  3. scores per (pair, k-tile) land in 2-bank PSUM tiles so the full
     valid q-row [ki*128, 1024) is contiguous; causal masking is a DVE
     add of a -1e30 triangle on the diagonal block; ONE Exp per
     (head, k-tile) with accum_out giving the full row sum for free.
  4. normalization folded into V (vsc = V * 1/rowsum);
     z^T[dh, q] = vsc.T @ attnT, two heads into one PSUM bank via
     column-group tiling.
  5. out[s, :] = z.T @ Wo^T accumulated over f tiles, DMA out per chunk.

All matmul operands are bfloat16.  The pair loop is software-pipelined:
the next pair's QKV projections are emitted between the scores and the
final z chunk so the PE always has independent work while ACT drains exps.
"""

import numpy as np
from contextlib import ExitStack

import concourse.bacc as bacc
import concourse.bass as bass
import concourse.tile as tile
from concourse import mybir
from concourse.bass_utils import run_bass_kernel_spmd

B, S, DM, H, DH = 8, 1024, 1024, 16, 64
P = 128
EPS = 1.1920929e-07
NEG = -1e30
F32 = mybir.dt.float32
BF16 = mybir.dt.bfloat16
NS = S // P      # 8 s-tiles (also k-tiles)
ND = DM // P     # 8 d-chunks
NPAIR = H // 2   # 8 head pairs
QCH = 512        # one PSUM bank of fp32
NQC = S // QCH   # 2 q chunks

MMDT = BF16
PDT = BF16


def build_program(with_bias=False):
    nc = bacc.Bacc("TRN2", target_bir_lowering=False, debug=False)

    xb = nc.dram_tensor("xb", [S, DM], PDT, kind="ExternalInput").ap()
    # (ft, dd, dk, f): ft 0-7 = Q pair tiles, 8-15 = K pair tiles
    wqk = nc.dram_tensor("wqk", [2 * NPAIR, P, ND, P], PDT, kind="ExternalInput").ap()
    wv = nc.dram_tensor("wv", [ND, P, DM], PDT, kind="ExternalInput").ap()
    wo = nc.dram_tensor("wo", [ND, P, DM], PDT, kind="ExternalInput").ap()
    ident = nc.dram_tensor("ident", [P, P], PDT, kind="ExternalInput").ap()
    trimask = nc.dram_tensor("trimask", [P, P], F32, kind="ExternalInput").ap()
    bqk = bv = None
    if with_bias:
        bqk = nc.dram_tensor("bqk", [P, 2 * NPAIR], F32, kind="ExternalInput").ap()
        bv = nc.dram_tensor("bv", [1, DM], PDT, kind="ExternalInput").ap()
    out = nc.dram_tensor("out", [S, DM], F32, kind="ExternalOutput").ap()

    with tile.TileContext(nc) as tc:
        with ExitStack() as ctx:
            _build_body(ctx, tc, xb, wqk, wv, wo, ident, trimask, bqk, bv, out)
    nc.compile()
    return nc


def _build_body(ctx, tc, xb, wqk, wv, wo, ident, trimask, bqk, bv, out):
    nc = tc.nc
    AF = mybir.ActivationFunctionType
    ALU = mybir.AluOpType

    singles = ctx.enter_context(tc.tile_pool(name="singles", bufs=1))
    big8 = ctx.enter_context(tc.tile_pool(name="big8", bufs=1))    # V tiles
    xqp = ctx.enter_context(tc.tile_pool(name="xqp", bufs=1))      # x natural
    xbfp = ctx.enter_context(tc.tile_pool(name="xbfp", bufs=2))    # scaled bf16 x
    scr = ctx.enter_context(tc.tile_pool(name="scr", bufs=2))      # out tiles
    nt = ctx.enter_context(tc.tile_pool(name="nt", bufs=1))        # normT
    w8 = ctx.enter_context(tc.tile_pool(name="w8", bufs=1))        # wv then wo
    wqks = ctx.enter_context(tc.tile_pool(name="wqks", bufs=1))    # wqk stream
    qkp = ctx.enter_context(tc.tile_pool(name="qkp", bufs=3))      # qt/kt tiles
    attnp = ctx.enter_context(tc.tile_pool(name="attnp", bufs=2))  # exp(scores)
    ztp = ctx.enter_context(tc.tile_pool(name="ztp", bufs=1))      # z transposed
    vscp = ctx.enter_context(tc.tile_pool(name="vscp", bufs=2))    # scaled V
    sm = ctx.enter_context(tc.tile_pool(name="sm", bufs=3))        # small stats
    ps = ctx.enter_context(tc.tile_pool(name="ps", bufs=1, space="PSUM"))

    # PSUM budget (8 banks): tag "sc" = [P, 1024] f32 (2 banks) x bufs 2
    # (scores only), tag "mm" = [P, 512] x bufs 2 (projection chunks),
    # tag "z" = [P, 512] x bufs 2 (z accum, transposes, last out tile).
    def sc_tile(name):
        return ps.tile([P, 2 * QCH], F32, tag="sc", bufs=2, name=name)

    def mm_bank(name):
        return ps.tile([P, QCH], F32, tag="mm", bufs=2, name=name)

    def z_bank(name):
        return ps.tile([P, QCH], F32, tag="z", bufs=2, name=name)

    # ---------------- Phase A: RMSNorm + transpose ----------------
    # x tile 0 heads the RMS->transpose critical path, so its DMA goes
    # first; ident (needed by the first transpose) right after.
    x_qs = []
    for qi in range(NS // 2):
        x_q = xqp.tile([P, 2, DM], PDT, tag=f"xq{qi}", name=f"xq{qi}")
        x_qs.append(x_q)
    nc.sync.dma_start(out=x_qs[0][:, 0, :], in_=xb[0:P, :])
    ident_sb = singles.tile([P, P], PDT, tag="ident")
    nc.sync.dma_start(out=ident_sb, in_=ident)
    nc.sync.dma_start(out=x_qs[0][:, 1, :], in_=xb[P:2 * P, :])
    for qi in range(1, NS // 2):
        nc.sync.dma_start(
            out=x_qs[qi],
            in_=xb[qi * 2 * P:(qi + 1) * 2 * P, :].rearrange(
                "(j p) d -> p j d", j=2))
    eps_sb = singles.tile([P, 1], F32, tag="eps")
    nc.vector.memset(eps_sb, EPS)
    # touch Sqrt and Exp now so their ACT table loads (~1.3us each) happen
    # during the input DMA instead of on the first real use
    warm_sb = singles.tile([P, 1], F32, tag="warm")
    nc.scalar.activation(out=warm_sb, in_=eps_sb, func=AF.Sqrt)
    nc.scalar.activation(out=warm_sb, in_=eps_sb, func=AF.Exp)
    tri_sb = singles.tile([P, P], F32, tag="tri")
    nc.sync.dma_start(out=tri_sb, in_=trimask)
    wqk_pre = {}
    for ft in (0, NPAIR, 1, NPAIR + 1):
        w_t = wqks.tile([P, ND, P], PDT, tag="wqk", bufs=4, name=f"wqk{ft}")
        nc.sync.dma_start(out=w_t, in_=wqk[ft])
        wqk_pre[ft] = w_t
    bqk_sb = bv_sb = ones_sb = None
    if bqk is not None:
        bqk_sb = singles.tile([P, 2 * NPAIR], F32, tag="bqk")
        nc.sync.dma_start(out=bqk_sb, in_=bqk)
        bv_sb = singles.tile([1, DM], PDT, tag="bv")
        nc.sync.dma_start(out=bv_sb, in_=bv)
        ones_sb = singles.tile([1, P], PDT, tag="ones")
        nc.vector.memset(ones_sb, 1.0)

    # unified normT: nt_all[:, dk, s]
    nt_all = nt.tile([P, ND, S], PDT, tag="nt", name="normT")

    # ---------------- QK projection helpers ----------------
    # A (which, chunk) group is 8 accumulating matmuls into one mm bank,
    # cast to its half of the qkT destination tile.  Groups are emitted
    # either whole (phase A/B) or sliced 2-dk at a time into the pair
    # loop so the PE has work while ACT drains exps.
    qt_tiles = {}
    kt_tiles = {}

    def dma_wqk(t):
        for ft in (t, NPAIR + t):
            if ft not in wqk_pre:
                w_t = wqks.tile([P, ND, P], PDT, tag="wqk", bufs=4,
                                name=f"wqk{ft}")
                nc.sync.dma_start(out=w_t, in_=wqk[ft])
                wqk_pre[ft] = w_t

    def qk_alloc(t, which):
        tiles = qt_tiles if which == "qt" else kt_tiles
        ft = t if which == "qt" else NPAIR + t
        w_t = wqk_pre.pop(ft)
        dst = qkp.tile([P, S], PDT, tag=which, name=f"{which}{t}")
        tiles[t] = (dst, w_t)
        return dst, w_t

    def qk_mms(t, which, sc, mm, dks):
        w_t = (qt_tiles if which == "qt" else kt_tiles)[t][1]
        for dk in dks:
            nc.tensor.matmul(
                mm,
                w_t[:, dk, :],
                nt_all[:, dk, sc * QCH:(sc + 1) * QCH],
                start=(dk == 0), stop=(dk == ND - 1),
            )

    def qk_cast(t, which, sc, mm):
        dst = (qt_tiles if which == "qt" else kt_tiles)[t][0]
        ft = t if which == "qt" else NPAIR + t
        dv = dst[:, sc * QCH:(sc + 1) * QCH]
        if bqk_sb is not None:
            nc.vector.tensor_scalar_add(out=dv, in0=mm,
                                        scalar1=bqk_sb[:, ft:ft + 1])
        else:
            nc.vector.tensor_copy(out=dv, in_=mm)

    def emit_qk_group(t, which, sc):
        if sc == 0 and t not in (qt_tiles if which == "qt" else kt_tiles):
            qk_alloc(t, which)
        mm = mm_bank(f"qk{which}{t}_{sc}")
        qk_mms(t, which, sc, mm, range(ND))
        qk_cast(t, which, sc, mm)

    for st in range(NS):
        x_t = x_qs[st // 2][:, st % 2, :]
        sqscr = sm.tile([P, DM], PDT, tag="sqscr", bufs=1, name=f"sqscr{st}")
        ssum = sm.tile([P, 1], F32, tag="ssA", name=f"ssA{st}")
        nc.vector.scalar_tensor_tensor(
            out=sqscr, in0=x_t, scalar=1.0, in1=x_t,
            op0=ALU.mult, op1=ALU.mult, accum_out=ssum)
        rs_t = sm.tile([P, 1], F32, tag="rs", name=f"rs{st}")
        # rs = sqrt(mean(x^2) + eps) then reciprocal
        nc.scalar.activation(out=rs_t, in_=ssum, func=AF.Sqrt,
                             bias=eps_sb, scale=1.0 / DM)
        nc.vector.reciprocal(out=rs_t, in_=rs_t)
        xbf = xbfp.tile([P, DM], PDT, tag="xbf", name=f"xbf{st}")
        nc.vector.tensor_scalar_mul(out=xbf, in0=x_t, scalar1=rs_t)
        # batched PE transposes: 4 per PSUM bank, one wide cast to normT
        for g in range(2):
            tp_ps = ps.tile([P, 4, P], PDT, tag="z", bufs=2, name=f"tp{st}_{g}")
            for j in range(4):
                dk = 4 * g + j
                nc.tensor.transpose(tp_ps[:, j, 0:P],
                                    xbf[:, dk * P:(dk + 1) * P], ident_sb)
            nc.scalar.copy(
                out=nt_all[:, 4 * g:4 * g + 4, st * P:(st + 1) * P],
                in_=tp_ps[:, :, 0:P])
        # normT cols 0..511 complete: give the PE pair-0 projection work so
        # it isn't stuck behind transposes that wait on the scalar RMS chain
        if st == 3:
            emit_qk_group(0, "qt", 0)
        if st == 5:
            emit_qk_group(0, "kt", 0)

    # ---------------- Phase B: rest of QK pairs 0,1 then V ----------------
    emit_qk_group(0, "qt", 1)
    emit_qk_group(0, "kt", 1)
    for which in ("qt", "kt"):
        qk_alloc(1, which)
        for sc in range(NQC):
            emit_qk_group(1, which, sc)

    wv_sb = []
    for dk in range(ND):
        w_t = w8.tile([P, DM], PDT, tag=f"w{dk}", name=f"wv{dk}")
        nc.sync.dma_start(out=w_t, in_=wv[dk])
        wv_sb.append(w_t)
    vs = []
    for st in range(NS):
        v_t = big8.tile([P, DM], PDT, tag=f"b{st}", name=f"vs{st}")
        vs.append(v_t)
        mm = sc_tile(f"vps{st}")
        for dk in range(ND):
            for fvc in range(NQC):
                nc.tensor.matmul(
                    mm[:, fvc * QCH:(fvc + 1) * QCH],
                    nt_all[:, dk, st * P:(st + 1) * P],
                    wv_sb[dk][:, fvc * QCH:(fvc + 1) * QCH],
                    start=(dk == 0),
                    stop=(dk == ND - 1 and bv_sb is None),
                )
        if bv_sb is not None:
            for fvc in range(NQC):
                nc.tensor.matmul(
                    mm[:, fvc * QCH:(fvc + 1) * QCH], ones_sb,
                    bv_sb[:, fvc * QCH:(fvc + 1) * QCH],
                    start=False, stop=True,
                )
        nc.vector.tensor_copy(
            out=v_t.rearrange("p (c q) -> p c q", c=2),
            in_=mm.rearrange("p (c q) -> p c q", c=2))

    # prefetch Wo early: the w8 slots free up as soon as V consumed wv
    wo_sb = []
    for fk in range(ND):
        w_t = w8.tile([P, DM], PDT, tag=f"w{fk}", name=f"wo{fk}")
        nc.sync.dma_start(out=w_t, in_=wo[fk])
        wo_sb.append(w_t)

    # ---------------- Phase E emitter (out projection per s-tile) --------
    zT = []

    def emit_out_tile(st):
        o_t = scr.tile([P, DM], F32, tag="osb", name=f"osb{st}")
        if st == NS - 1:
            # quarter-width groups on z banks: the copy+DMA chain overlaps
            # the remaining matmuls instead of serializing at kernel end
            w = QCH // 2
            for half in range(2):
                mm = z_bank(f"ops{st}_{half}")
                for sub in range(2):
                    dmc = 2 * half + sub
                    for fk in range(ND):
                        nc.tensor.matmul(
                            mm[:, sub * w:(sub + 1) * w],
                            zT[fk][:, st * P:(st + 1) * P],
                            wo_sb[fk][:, dmc * w:(dmc + 1) * w],
                            start=(fk == 0), stop=(fk == ND - 1),
                        )
                    nc.vector.tensor_copy(
                        out=o_t[:, dmc * w:(dmc + 1) * w],
                        in_=mm[:, sub * w:(sub + 1) * w])
                    nc.sync.dma_start(
                        out=out[st * P:(st + 1) * P, dmc * w:(dmc + 1) * w],
                        in_=o_t[:, dmc * w:(dmc + 1) * w])
            return
        mm = sc_tile(f"ops{st}")
        for fk in range(ND):
            for dmc in range(NQC):
                nc.tensor.matmul(
                    mm[:, dmc * QCH:(dmc + 1) * QCH],
                    zT[fk][:, st * P:(st + 1) * P],
                    wo_sb[fk][:, dmc * QCH:(dmc + 1) * QCH],
                    start=(fk == 0), stop=(fk == ND - 1),
                )
        for dmc in range(NQC):
            nc.vector.tensor_copy(out=o_t[:, dmc * QCH:(dmc + 1) * QCH],
                                  in_=mm[:, dmc * QCH:(dmc + 1) * QCH])
            nc.sync.dma_start(
                out=out[st * P:(st + 1) * P, dmc * QCH:(dmc + 1) * QCH],
                in_=o_t[:, dmc * QCH:(dmc + 1) * QCH])

    # ---------------- Phase C/D: attention per head pair ----------------
    for t in range(NPAIR):
        have_proj = t + 2 < NPAIR
        if have_proj:
            dma_wqk(t + 2)
            qk_alloc(t + 2, "qt")
            qk_alloc(t + 2, "kt")
            proj_mm = {}
        qt, kt = qt_tiles.pop(t)[0], kt_tiles.pop(t)[0]

        def emit_proj_slice(ki, t=t):
            """2 dk-steps of the t+2 projection, interleaved into the ki
            loop: Q during kis 0-3, K during kis 4-7.  One LDWEIGHTS per
            dk serves both 512-wide chunks."""
            if not have_proj:
                return
            which = "qt" if ki < 4 else "kt"
            if ki % 4 == 0:
                proj_mm[(which, 0)] = mm_bank(f"qk{which}{t + 2}_0")
                proj_mm[(which, 1)] = mm_bank(f"qk{which}{t + 2}_1")
            w_t = (qt_tiles if which == "qt" else kt_tiles)[t + 2][1]
            for dk in (2 * (ki % 4), 2 * (ki % 4) + 1):
                for sc in range(NQC):
                    nc.tensor.matmul(
                        proj_mm[(which, sc)],
                        w_t[:, dk, :],
                        nt_all[:, dk, sc * QCH:(sc + 1) * QCH],
                        start=(dk == 0), stop=(dk == ND - 1),
                    )
            if ki == 3:
                for sc in range(NQC):
                    qk_cast(t + 2, "qt", sc, proj_mm[("qt", sc)])

        z_t = ztp.tile([P, S], PDT, tag=f"zt{t}", name=f"zT{t}")
        zT.append(z_t)
        attn = {}   # (head_local, ki) -> sbuf tile [P, width]
        vsc_d = {}  # ki -> scaled V slice [P, 128] for this pair
        rsp_d = {}  # ki -> row sums [P, 2] (hl 0, 1)

        def emit_ri_vsc(ki, t=t, vsc_d=vsc_d, rsp_d=rsp_d):
            ri = sm.tile([P, 2], F32, tag="ri", name=f"ri{t}_{ki}")
            nc.vector.reciprocal(out=ri, in_=rsp_d[ki])
            vsc = vscp.tile([P, P], MMDT, tag=f"vsc{ki}", name=f"vsc{t}_{ki}")
            vsc_d[ki] = vsc
            ri_b = bass.AP(tensor=ri.tensor, offset=ri.offset,
                           ap=[list(ri.ap[0]), list(ri.ap[1]), [0, DH]])
            nc.vector.tensor_tensor(
                out=vsc.rearrange("p (h d) -> p h d", h=2),
                in0=vs[ki][:, t * P:(t + 1) * P].rearrange(
                    "p (h d) -> p h d", h=2),
                in1=ri_b, op=ALU.mult)

        z_ps = {}
        for qc in range(NQC):
            z_ps[qc] = z_bank(f"zps{t}_{qc}")

        def emit_z_contrib(ki, last1=False, t=t, attn=attn, vsc_d=vsc_d,
                           z_ps=z_ps):
            chunks = ([(0, ki == 3)] if ki < 4 else []) + [(1, last1)]
            for qc, stop in chunks:
                q0 = max(qc * QCH, ki * P)
                for hl in (0, 1):
                    nc.tensor.matmul(
                        z_ps[qc][hl * DH:(hl + 1) * DH, q0 - qc * QCH:QCH],
                        vsc_d[ki][:, hl * DH:(hl + 1) * DH],
                        attn[(hl, ki)][:, q0 - ki * P:(qc + 1) * QCH - ki * P],
                        start=(ki == 0), stop=stop,
                    )

        def finish_z_chunk(qc, t=t, z_t=z_t, z_ps=z_ps):
            # chunk 0's cast goes to scalar (emitted after the pair's last
            # exp, so it can't delay them); chunk 1's to vector
            if qc == 0 and t != NPAIR - 1:
                nc.scalar.copy(out=z_t[:, qc * QCH:(qc + 1) * QCH],
                               in_=z_ps[qc])
            else:
                nc.vector.tensor_copy(
                    out=z_t[:, qc * QCH:(qc + 1) * QCH], in_=z_ps[qc])

        for ki in range(NS):
            width = S - ki * P
            rsp = sm.tile([P, 2], F32, tag="rsp", name=f"rsp{t}_{ki}")
            rsp_d[ki] = rsp
            if ki < 4:
                # two 2-bank tiles, one per head; each holds the full
                # valid q span [ki*128, 1024) contiguously
                s_hl = [sc_tile(f"sps{t}_{hl}_{ki}") for hl in (0, 1)]
                for qc in range(NQC):
                    q0 = max(qc * QCH, ki * P)
                    for hl, prange in ((0, slice(0, DH)), (1, slice(DH, P))):
                        nc.tensor.matmul(
                            s_hl[hl][:, q0:(qc + 1) * QCH],
                            kt[prange, ki * P:(ki + 1) * P],
                            qt[prange, q0:(qc + 1) * QCH],
                            start=True, stop=True,
                        )
                for hl in (0, 1):
                    nc.vector.tensor_add(
                        out=s_hl[hl][:, ki * P:(ki + 1) * P],
                        in0=s_hl[hl][:, ki * P:(ki + 1) * P], in1=tri_sb)
                for hl in (0, 1):
                    a_t = attnp.tile([P, width], MMDT, tag=f"at{ki}",
                                     name=f"attn{t}_{hl}_{ki}")
                    attn[(hl, ki)] = a_t
                    nc.scalar.activation(
                        out=a_t, in_=s_hl[hl][:, ki * P:2 * QCH],
                        func=AF.Exp, accum_out=rsp[:, hl:hl + 1])
            else:
                # one 2-bank tile: bank 0 = head 0, bank 1 = head 1
                s01 = sc_tile(f"sps{t}_{ki}")
                off = ki * P - QCH
                for hl, prange in ((0, slice(0, DH)), (1, slice(DH, P))):
                    nc.tensor.matmul(
                        s01[:, hl * QCH + off:(hl + 1) * QCH],
                        kt[prange, ki * P:(ki + 1) * P],
                        qt[prange, ki * P:S],
                        start=True, stop=True,
                    )
                for hl in (0, 1):
                    nc.vector.tensor_add(
                        out=s01[:, hl * QCH + off:hl * QCH + off + P],
                        in0=s01[:, hl * QCH + off:hl * QCH + off + P],
                        in1=tri_sb)
                for hl in (0, 1):
                    a_t = attnp.tile([P, width], MMDT, tag=f"at{ki}",
                                     name=f"attn{t}_{hl}_{ki}")
                    attn[(hl, ki)] = a_t
                    nc.scalar.activation(
                        out=a_t, in_=s01[:, hl * QCH + off:(hl + 1) * QCH],
                        func=AF.Exp, accum_out=rsp[:, hl:hl + 1])
            # next pair's projection slice: PE work while ACT drains exps
            emit_proj_slice(ki)
            # recip+vsc for the PREVIOUS ki (keeps the in-order vector
            # queue from gating the next exp)
            if ki >= 1:
                emit_ri_vsc(ki - 1)
            if ki >= 3:
                emit_z_contrib(ki - 3)
        finish_z_chunk(0)
        if have_proj:
            for sc in range(NQC):
                qk_cast(t + 2, "kt", sc, proj_mm[("kt", sc)])
        if t == NPAIR - 1:
            for st in range(4):
                emit_out_tile(st)
            emit_ri_vsc(NS - 1)
            emit_z_contrib(NS - 3)
            emit_z_contrib(NS - 2)
            emit_z_contrib(NS - 1, last1=True)
            finish_z_chunk(1)
            for st in range(4, NS):
                emit_out_tile(st)
        else:
            emit_ri_vsc(NS - 1)
            emit_z_contrib(NS - 3)
            emit_z_contrib(NS - 2)
            emit_z_contrib(NS - 1, last1=True)
            finish_z_chunk(1)


def prep_inputs(x, W_qkv, b_qkv):
    """Host-side re-layout of inputs (weights de-interleave/transpose/tile)."""
    x = np.ascontiguousarray(np.asarray(x, np.float32)).astype(NP_PDT)
    W = np.asarray(W_qkv, np.float32).reshape(H, DH, 3, DM)
    Wq = W[:, :, 0, :].reshape(H * DH, DM)
    Wk = W[:, :, 1, :].reshape(H * DH, DM)
    Wv = W[:, :, 2, :].reshape(H * DH, DM)
    WqkT = np.ascontiguousarray(np.concatenate([Wq, Wk], 0).T)   # [DM, 2048]
    wqk_host = np.ascontiguousarray(
        WqkT.reshape(ND, P, 2 * NPAIR, P).transpose(2, 1, 0, 3)).astype(NP_PDT)
    wv_host = np.ascontiguousarray(Wv.T).reshape(ND, P, DM).astype(NP_PDT)
    ident = np.eye(P, dtype=np.float32).astype(NP_PDT)
    idx = np.arange(P)
    trimask = np.where(idx[None, :] >= idx[:, None], 0.0, NEG).astype(np.float32)

    b = np.asarray(b_qkv, np.float32).reshape(H, DH, 3)
    bq = b[:, :, 0].reshape(H * DH)
    bk = b[:, :, 1].reshape(H * DH)
    bvv = b[:, :, 2].reshape(H * DH)
    bqk_host = np.ascontiguousarray(
        np.concatenate([bq, bk]).reshape(2 * NPAIR, P).T)         # [P, 16]
    return x, wqk_host, wv_host, ident, trimask, bqk_host, bvv


import ml_dtypes

NP_PDT = ml_dtypes.bfloat16

_prog_cache = {}


def kernel(x, W_qkv, b_qkv, W_o, b_o, trace=False):
    x, wqk_host, wv_host, ident, trimask, bqk_host, bvv = prep_inputs(
        x, W_qkv, b_qkv)
    wo_host = np.ascontiguousarray(np.asarray(W_o, np.float32).T).reshape(ND, P, DM).astype(NP_PDT)
    with_bias = bool(np.any(np.asarray(b_qkv)))
    key = with_bias
    if key not in _prog_cache:
        _prog_cache[key] = build_program(with_bias=with_bias)
    nc = _prog_cache[key]

    in_maps = []
    for bi in range(B):
        m = {
            "xb": x[bi], "wqk": wqk_host, "wv": wv_host, "wo": wo_host,
            "ident": ident, "trimask": trimask,
        }
        if with_bias:
            m["bqk"] = bqk_host
            m["bv"] = bvv.reshape(1, DM).astype(NP_PDT)
        in_maps.append(m)

    res = run_bass_kernel_spmd(nc, in_maps, core_ids=list(range(B)), trace=trace)
    out = np.stack([res.results[bi]["out"] for bi in range(B)]).astype(np.float32)
    out += np.asarray(b_o, np.float32)[None, None, :]
    if trace:
        kernel.last_results = res
    return out


# revision 17
# speedup vs baseline: 1.2211x; 1.0452x over previous
"""Trainium2 Bass kernel for nn_AttentionBlock (RMSNorm + QKV + causal
attention with softmax over the QUERY axis + output projection).

Sharding: data-parallel over batch. B=8 -> one batch element per NeuronCore,
no collectives. Weights are re-laid-out on the host (de-interleave the
(h, dh, 3) QKV packing, transpose so the contraction dim d lands on SBUF
partitions, and pre-tile so every DMA is contiguous).

Device dataflow per core (S=1024, D=1024, H=16, Dh=64):
  1. x natural [s,d] tiles (bf16) -> sum(x^2) via DVE multiply+accum ->
     rsqrt scale -> xbf (bf16) -> PE-transpose (batched 4 per PSUM bank)
     -> normT [d, s] in one unified SBUF tile.
  2. qkT[f, s] = Wqk^T.T @ normT (a 128-row f-tile holds a HEAD PAIR);
     V[s, f] = normT.T @ Wv^T.  Projection loops run the weight tile
     (stationary operand) in the OUTER loop over both 512-wide chunks so
     each LDWEIGHTS serves two matmuls; each projection tile drains its
     2-bank PSUM tile with a single DVE cast.
  3. scores per (pair, k-tile) land in 2-bank PSUM tiles so the full
     valid q-row [ki*128, 1024) is contiguous; causal masking is a DVE
     add of a -1e30 triangle on the diagonal block; ONE Exp per
     (head, k-tile) with accum_out giving the full row sum for free.
  4. normalization folded into V (vsc = V * 1/rowsum);
     z^T[dh, q] = vsc.T @ attnT, two heads into one PSUM bank via
     column-group tiling.
  5. out[s, :] = z.T @ Wo^T accumulated over f tiles, DMA out per chunk.

All matmul operands are bfloat16.  The pair loop is software-pipelined:
the next pair's QKV projections are emitted between the scores and the
final z chunk so the PE always has independent work while ACT drains exps.
"""

import numpy as np
from contextlib import ExitStack

import concourse.bacc as bacc
import concourse.bass as bass
import concourse.tile as tile
from concourse import mybir
from concourse.bass_utils import run_bass_kernel_spmd

B, S, DM, H, DH = 8, 1024, 1024, 16, 64
P = 128
EPS = 1.1920929e-07
NEG = -1e30
F32 = mybir.dt.float32
BF16 = mybir.dt.bfloat16
NS = S // P      # 8 s-tiles (also k-tiles)
ND = DM // P     # 8 d-chunks
NPAIR = H // 2   # 8 head pairs
QCH = 512        # one PSUM bank of fp32
NQC = S // QCH   # 2 q chunks

MMDT = BF16
PDT = BF16


def build_program(with_bias=False):
    nc = bacc.Bacc("TRN2", target_bir_lowering=False, debug=False)

    xb = nc.dram_tensor("xb", [S, DM], PDT, kind="ExternalInput").ap()
    # (ft, dd, dk, f): ft 0-7 = Q pair tiles, 8-15 = K pair tiles
    wqk = nc.dram_tensor("wqk", [2 * NPAIR, P, ND, P], PDT, kind="ExternalInput").ap()
    wv = nc.dram_tensor("wv", [ND, P, DM], PDT, kind="ExternalInput").ap()
    wo = nc.dram_tensor("wo", [ND, P, DM], PDT, kind="ExternalInput").ap()
    ident = nc.dram_tensor("ident", [P, P], PDT, kind="ExternalInput").ap()
    trimask = nc.dram_tensor("trimask", [P, P], F32, kind="ExternalInput").ap()
    bqk = bv = None
    if with_bias:
        bqk = nc.dram_tensor("bqk", [P, 2 * NPAIR], F32, kind="ExternalInput").ap()
        bv = nc.dram_tensor("bv", [1, DM], PDT, kind="ExternalInput").ap()
    out = nc.dram_tensor("out", [S, DM], F32, kind="ExternalOutput").ap()

    with tile.TileContext(nc) as tc:
        with ExitStack() as ctx:
            _build_body(ctx, tc, xb, wqk, wv, wo, ident, trimask, bqk, bv, out)
    nc.compile()
    return nc


def _build_body(ctx, tc, xb, wqk, wv, wo, ident, trimask, bqk, bv, out):
    nc = tc.nc
    AF = mybir.ActivationFunctionType
    ALU = mybir.AluOpType

    singles = ctx.enter_context(tc.tile_pool(name="singles", bufs=1))
    big8 = ctx.enter_context(tc.tile_pool(name="big8", bufs=1))    # V tiles
    xqp = ctx.enter_context(tc.tile_pool(name="xqp", bufs=1))      # x natural
    xbfp = ctx.enter_context(tc.tile_pool(name="xbfp", bufs=2))    # scaled bf16 x
    scr = ctx.enter_context(tc.tile_pool(name="scr", bufs=2))      # out tiles
    nt = ctx.enter_context(tc.tile_pool(name="nt", bufs=1))        # normT
    w8 = ctx.enter_context(tc.tile_pool(name="w8", bufs=1))        # wv then wo
    wqks = ctx.enter_context(tc.tile_pool(name="wqks", bufs=1))    # wqk stream
    qkp = ctx.enter_context(tc.tile_pool(name="qkp", bufs=3))      # qt/kt tiles
    attnp = ctx.enter_context(tc.tile_pool(name="attnp", bufs=2))  # exp(scores)
    ztp = ctx.enter_context(tc.tile_pool(name="ztp", bufs=1))      # z transposed
    vscp = ctx.enter_context(tc.tile_pool(name="vscp", bufs=2))    # scaled V
    sm = ctx.enter_context(tc.tile_pool(name="sm", bufs=3))        # small stats
    ps = ctx.enter_context(tc.tile_pool(name="ps", bufs=1, space="PSUM"))

    # PSUM budget (8 banks): tag "sc" = [P, 1024] f32 (2 banks) x bufs 2
    # (scores only), tag "mm" = [P, 512] x bufs 2 (projection chunks),
    # tag "z" = [P, 512] x bufs 2 (z accum, transposes, last out tile).
    def sc_tile(name):
        return ps.tile([P, 2 * QCH], F32, tag="sc", bufs=2, name=name)

    def mm_bank(name):
        return ps.tile([P, QCH], F32, tag="mm", bufs=2, name=name)

    def z_bank(name):
        return ps.tile([P, QCH], F32, tag="z", bufs=2, name=name)

    # ---------------- Phase A: RMSNorm + transpose ----------------
    # x tile 0 heads the RMS->transpose critical path, so its DMA goes
    # first; ident (needed by the first transpose) right after.
    x_qs = []
    for qi in range(NS // 2):
        x_q = xqp.tile([P, 2, DM], PDT, tag=f"xq{qi}", name=f"xq{qi}")
        x_qs.append(x_q)
    nc.sync.dma_start(out=x_qs[0][:, 0, :], in_=xb[0:P, :])
    ident_sb = singles.tile([P, P], PDT, tag="ident")
    nc.sync.dma_start(out=ident_sb, in_=ident)
    nc.sync.dma_start(out=x_qs[0][:, 1, :], in_=xb[P:2 * P, :])
    for qi in range(1, NS // 2):
        nc.sync.dma_start(
            out=x_qs[qi],
            in_=xb[qi * 2 * P:(qi + 1) * 2 * P, :].rearrange(
                "(j p) d -> p j d", j=2))
    eps_sb = singles.tile([P, 1], F32, tag="eps")
    nc.vector.memset(eps_sb, EPS)
    # touch Sqrt and Exp now so their ACT table loads (~1.3us each) happen
    # during the input DMA instead of on the first real use
    warm_sb = singles.tile([P, 1], F32, tag="warm")
    nc.scalar.activation(out=warm_sb, in_=eps_sb, func=AF.Sqrt)
    nc.scalar.activation(out=warm_sb, in_=eps_sb, func=AF.Exp)
    tri_sb = singles.tile([P, P], F32, tag="tri")
    nc.sync.dma_start(out=tri_sb, in_=trimask)
    wqk_pre = {}
    for ft in (0, NPAIR, 1, NPAIR + 1):
        w_t = wqks.tile([P, ND, P], PDT, tag="wqk", bufs=4, name=f"wqk{ft}")
        nc.sync.dma_start(out=w_t, in_=wqk[ft])
        wqk_pre[ft] = w_t
    bqk_sb = bv_sb = ones_sb = None
    if bqk is not None:
        bqk_sb = singles.tile([P, 2 * NPAIR], F32, tag="bqk")
        nc.sync.dma_start(out=bqk_sb, in_=bqk)
        bv_sb = singles.tile([1, DM], PDT, tag="bv")
        nc.sync.dma_start(out=bv_sb, in_=bv)
        ones_sb = singles.tile([1, P], PDT, tag="ones")
        nc.vector.memset(ones_sb, 1.0)

    # unified normT: nt_all[:, dk, s]
    nt_all = nt.tile([P, ND, S], PDT, tag="nt", name="normT")

    # ---------------- QK projection helpers ----------------
    # A (which, chunk) group is 8 accumulating matmuls into one mm bank,
    # cast to its half of the qkT destination tile.  Groups are emitted
    # either whole (phase A/B) or sliced 2-dk at a time into the pair
    # loop so the PE has work while ACT drains exps.
    qt_tiles = {}
    kt_tiles = {}

    def dma_wqk(t):
        for ft in (t, NPAIR + t):
            if ft not in wqk_pre:
                w_t = wqks.tile([P, ND, P], PDT, tag="wqk", bufs=4,
                                name=f"wqk{ft}")
                nc.sync.dma_start(out=w_t, in_=wqk[ft])
                wqk_pre[ft] = w_t

    def qk_alloc(t, which):
        tiles = qt_tiles if which == "qt" else kt_tiles
        ft = t if which == "qt" else NPAIR + t
        w_t = wqk_pre.pop(ft)
        dst = qkp.tile([P, S], PDT, tag=which, name=f"{which}{t}")
        tiles[t] = (dst, w_t)
        return dst, w_t

    def qk_mms(t, which, sc, mm, dks):
        w_t = (qt_tiles if which == "qt" else kt_tiles)[t][1]
        for dk in dks:
            nc.tensor.matmul(
                mm,
                w_t[:, dk, :],
                nt_all[:, dk, sc * QCH:(sc + 1) * QCH],
                start=(dk == 0), stop=(dk == ND - 1),
            )

    def qk_cast(t, which, sc, mm):
        dst = (qt_tiles if which == "qt" else kt_tiles)[t][0]
        ft = t if which == "qt" else NPAIR + t
        dv = dst[:, sc * QCH:(sc + 1) * QCH]
        if bqk_sb is not None:
            nc.vector.tensor_scalar_add(out=dv, in0=mm,
                                        scalar1=bqk_sb[:, ft:ft + 1])
        else:
            nc.vector.tensor_copy(out=dv, in_=mm)

    def emit_qk_group(t, which, sc):
        if sc == 0 and t not in (qt_tiles if which == "qt" else kt_tiles):
            qk_alloc(t, which)
        mm = mm_bank(f"qk{which}{t}_{sc}")
        qk_mms(t, which, sc, mm, range(ND))
        qk_cast(t, which, sc, mm)

    for st in range(NS):
        x_t = x_qs[st // 2][:, st % 2, :]
        sqscr = sm.tile([P, DM], PDT, tag="sqscr", bufs=1, name=f"sqscr{st}")
        ssum = sm.tile([P, 1], F32, tag="ssA", name=f"ssA{st}")
        nc.vector.scalar_tensor_tensor(
            out=sqscr, in0=x_t, scalar=1.0, in1=x_t,
            op0=ALU.mult, op1=ALU.mult, accum_out=ssum)
        rs_t = sm.tile([P, 1], F32, tag="rs", name=f"rs{st}")
        # rs = sqrt(mean(x^2) + eps) then reciprocal
        nc.scalar.activation(out=rs_t, in_=ssum, func=AF.Sqrt,
                             bias=eps_sb, scale=1.0 / DM)
        nc.vector.reciprocal(out=rs_t, in_=rs_t)
        xbf = xbfp.tile([P, DM], PDT, tag="xbf", name=f"xbf{st}")
        nc.vector.tensor_scalar_mul(out=xbf, in0=x_t, scalar1=rs_t)
        # batched PE transposes: 4 per PSUM bank, one wide cast to normT
        for g in range(2):
            tp_ps = ps.tile([P, 4, P], PDT, tag="z", bufs=2, name=f"tp{st}_{g}")
            for j in range(4):
                dk = 4 * g + j
                nc.tensor.transpose(tp_ps[:, j, 0:P],
                                    xbf[:, dk * P:(dk + 1) * P], ident_sb)
            nc.scalar.copy(
                out=nt_all[:, 4 * g:4 * g + 4, st * P:(st + 1) * P],
                in_=tp_ps[:, :, 0:P])
        # normT cols 0..511 complete: give the PE pair-0 projection work so
        # it isn't stuck behind transposes that wait on the scalar RMS chain
        if st == 3:
            emit_qk_group(0, "qt", 0)
        if st == 5:
            emit_qk_group(0, "kt", 0)

    # ---------------- Phase B: rest of QK pair 0, then V ----------------
    # (pair 1's projection is interleaved into pair 0's ki loop)
    emit_qk_group(0, "qt", 1)
    emit_qk_group(0, "kt", 1)

    wv_sb = []
    for dk in range(ND):
        w_t = w8.tile([P, DM], PDT, tag=f"w{dk}", name=f"wv{dk}")
        nc.sync.dma_start(out=w_t, in_=wv[dk])
        wv_sb.append(w_t)
    vs = []
    for st in range(NS):
        v_t = big8.tile([P, DM], PDT, tag=f"b{st}", name=f"vs{st}")
        vs.append(v_t)
        mm = sc_tile(f"vps{st}")
        for dk in range(ND):
            for fvc in range(NQC):
                nc.tensor.matmul(
                    mm[:, fvc * QCH:(fvc + 1) * QCH],
                    nt_all[:, dk, st * P:(st + 1) * P],
                    wv_sb[dk][:, fvc * QCH:(fvc + 1) * QCH],
                    start=(dk == 0),
                    stop=(dk == ND - 1 and bv_sb is None),
                )
        if bv_sb is not None:
            for fvc in range(NQC):
                nc.tensor.matmul(
                    mm[:, fvc * QCH:(fvc + 1) * QCH], ones_sb,
                    bv_sb[:, fvc * QCH:(fvc + 1) * QCH],
                    start=False, stop=True,
                )
        nc.vector.tensor_copy(
            out=v_t.rearrange("p (c q) -> p c q", c=2),
            in_=mm.rearrange("p (c q) -> p c q", c=2))

    # prefetch Wo early: the w8 slots free up as soon as V consumed wv
    wo_sb = []
    for fk in range(ND):
        w_t = w8.tile([P, DM], PDT, tag=f"w{fk}", name=f"wo{fk}")
        nc.sync.dma_start(out=w_t, in_=wo[fk])
        wo_sb.append(w_t)

    # ---------------- Phase E emitter (out projection per s-tile) --------
    zT = []

    def emit_out_tile(st):
        o_t = scr.tile([P, DM], F32, tag="osb", name=f"osb{st}")
        if st == NS - 1:
            # quarter-width groups on z banks: the copy+DMA chain overlaps
            # the remaining matmuls instead of serializing at kernel end
            w = QCH // 2
            for half in range(2):
                mm = z_bank(f"ops{st}_{half}")
                for sub in range(2):
                    dmc = 2 * half + sub
                    for fk in range(ND):
                        nc.tensor.matmul(
                            mm[:, sub * w:(sub + 1) * w],
                            zT[fk][:, st * P:(st + 1) * P],
                            wo_sb[fk][:, dmc * w:(dmc + 1) * w],
                            start=(fk == 0), stop=(fk == ND - 1),
                        )
                    nc.vector.tensor_copy(
                        out=o_t[:, dmc * w:(dmc + 1) * w],
                        in_=mm[:, sub * w:(sub + 1) * w])
                    nc.sync.dma_start(
                        out=out[st * P:(st + 1) * P, dmc * w:(dmc + 1) * w],
                        in_=o_t[:, dmc * w:(dmc + 1) * w])
            return
        mm = sc_tile(f"ops{st}")
        for fk in range(ND):
            for dmc in range(NQC):
                nc.tensor.matmul(
                    mm[:, dmc * QCH:(dmc + 1) * QCH],
                    zT[fk][:, st * P:(st + 1) * P],
                    wo_sb[fk][:, dmc * QCH:(dmc + 1) * QCH],
                    start=(fk == 0), stop=(fk == ND - 1),
                )
        for dmc in range(NQC):
            nc.vector.tensor_copy(out=o_t[:, dmc * QCH:(dmc + 1) * QCH],
                                  in_=mm[:, dmc * QCH:(dmc + 1) * QCH])
            nc.sync.dma_start(
                out=out[st * P:(st + 1) * P, dmc * QCH:(dmc + 1) * QCH],
                in_=o_t[:, dmc * QCH:(dmc + 1) * QCH])

    # ---------------- Phase C/D: attention per head pair ----------------
    # dk-step schedule for the lag-1 interleaved projection: Q over kis
    # 0-2 (cast at 2), K over kis 3-5 (cast at 5) so the next pair's
    # scores never wait on a cast at the pair boundary.
    PROJ_SCHED = {0: ("qt", (0, 1, 2)), 1: ("qt", (3, 4, 5)),
                  2: ("qt", (6, 7)), 3: ("kt", (0, 1, 2)),
                  4: ("kt", (3, 4, 5)), 5: ("kt", (6, 7))}

    for t in range(NPAIR):
        have_proj = t + 1 < NPAIR
        if t + 2 < NPAIR:
            dma_wqk(t + 2)
        if have_proj:
            qk_alloc(t + 1, "qt")
            qk_alloc(t + 1, "kt")
            proj_mm = {}
        qt, kt = qt_tiles.pop(t)[0], kt_tiles.pop(t)[0]

        def emit_proj_slice(ki, t=t):
            """dk-steps of the t+1 projection, interleaved into the ki
            loop.  One LDWEIGHTS per dk serves both 512-wide chunks."""
            if not have_proj or ki not in PROJ_SCHED:
                return
            which, dks = PROJ_SCHED[ki]
            if dks[0] == 0:
                proj_mm[(which, 0)] = mm_bank(f"qk{which}{t + 1}_0")
                proj_mm[(which, 1)] = mm_bank(f"qk{which}{t + 1}_1")
            w_t = (qt_tiles if which == "qt" else kt_tiles)[t + 1][1]
            for dk in dks:
                for sc in range(NQC):
                    nc.tensor.matmul(
                        proj_mm[(which, sc)],
                        w_t[:, dk, :],
                        nt_all[:, dk, sc * QCH:(sc + 1) * QCH],
                        start=(dk == 0), stop=(dk == ND - 1),
                    )
            if dks[-1] == ND - 1:
                for sc in range(NQC):
                    qk_cast(t + 1, which, sc, proj_mm[(which, sc)])

        z_t = ztp.tile([P, S], PDT, tag=f"zt{t}", name=f"zT{t}")
        zT.append(z_t)
        attn = {}   # (head_local, ki) -> sbuf tile [P, width]
        vsc_d = {}  # ki -> scaled V slice [P, 128] for this pair
        rsp_d = {}  # ki -> row sums [P, 2] (hl 0, 1)

        def emit_ri_vsc(ki, t=t, vsc_d=vsc_d, rsp_d=rsp_d):
            ri = sm.tile([P, 2], F32, tag="ri", name=f"ri{t}_{ki}")
            nc.vector.reciprocal(out=ri, in_=rsp_d[ki])
            vsc = vscp.tile([P, P], MMDT, tag=f"vsc{ki}", name=f"vsc{t}_{ki}")
            vsc_d[ki] = vsc
            ri_b = bass.AP(tensor=ri.tensor, offset=ri.offset,
                           ap=[list(ri.ap[0]), list(ri.ap[1]), [0, DH]])
            nc.vector.tensor_tensor(
                out=vsc.rearrange("p (h d) -> p h d", h=2),
                in0=vs[ki][:, t * P:(t + 1) * P].rearrange(
                    "p (h d) -> p h d", h=2),
                in1=ri_b, op=ALU.mult)

        z_ps = {}
        for qc in range(NQC):
            z_ps[qc] = z_bank(f"zps{t}_{qc}")

        def emit_z_contrib(ki, last1=False, t=t, attn=attn, vsc_d=vsc_d,
                           z_ps=z_ps):
            chunks = ([(0, ki == 3)] if ki < 4 else []) + [(1, last1)]
            for qc, stop in chunks:
                q0 = max(qc * QCH, ki * P)
                for hl in (0, 1):
                    nc.tensor.matmul(
                        z_ps[qc][hl * DH:(hl + 1) * DH, q0 - qc * QCH:QCH],
                        vsc_d[ki][:, hl * DH:(hl + 1) * DH],
                        attn[(hl, ki)][:, q0 - ki * P:(qc + 1) * QCH - ki * P],
                        start=(ki == 0), stop=stop,
                    )

        def finish_z_chunk(qc, t=t, z_t=z_t, z_ps=z_ps):
            # chunk 0's cast goes to scalar (emitted after the pair's last
            # exp, so it can't delay them); chunk 1's to vector
            if qc == 0 and t != NPAIR - 1:
                nc.scalar.copy(out=z_t[:, qc * QCH:(qc + 1) * QCH],
                               in_=z_ps[qc])
            else:
                nc.vector.tensor_copy(
                    out=z_t[:, qc * QCH:(qc + 1) * QCH], in_=z_ps[qc])

        for ki in range(NS):
            width = S - ki * P
            rsp = sm.tile([P, 2], F32, tag="rsp", name=f"rsp{t}_{ki}")
            rsp_d[ki] = rsp
            if ki < 4:
                # two 2-bank tiles, one per head; each holds the full
                # valid q span [ki*128, 1024) contiguously
                s_hl = [sc_tile(f"sps{t}_{hl}_{ki}") for hl in (0, 1)]
                # hl outer: one LDWEIGHTS of the kt slice serves both qc
                for hl, prange in ((0, slice(0, DH)), (1, slice(DH, P))):
                    for qc in range(NQC):
                        q0 = max(qc * QCH, ki * P)
                        nc.tensor.matmul(
                            s_hl[hl][:, q0:(qc + 1) * QCH],
                            kt[prange, ki * P:(ki + 1) * P],
                            qt[prange, q0:(qc + 1) * QCH],
                            start=True, stop=True,
                        )
                for hl in (0, 1):
                    nc.vector.tensor_add(
                        out=s_hl[hl][:, ki * P:(ki + 1) * P],
                        in0=s_hl[hl][:, ki * P:(ki + 1) * P], in1=tri_sb)
                for hl in (0, 1):
                    a_t = attnp.tile([P, width], MMDT, tag=f"at{ki}",
                                     name=f"attn{t}_{hl}_{ki}")
                    attn[(hl, ki)] = a_t
                    nc.scalar.activation(
                        out=a_t, in_=s_hl[hl][:, ki * P:2 * QCH],
                        func=AF.Exp, accum_out=rsp[:, hl:hl + 1])
            else:
                # one 2-bank tile: bank 0 = head 0, bank 1 = head 1
                s01 = sc_tile(f"sps{t}_{ki}")
                off = ki * P - QCH
                for hl, prange in ((0, slice(0, DH)), (1, slice(DH, P))):
                    nc.tensor.matmul(
                        s01[:, hl * QCH + off:(hl + 1) * QCH],
                        kt[prange, ki * P:(ki + 1) * P],
                        qt[prange, ki * P:S],
                        start=True, stop=True,
                    )
                for hl in (0, 1):
                    nc.vector.tensor_add(
                        out=s01[:, hl * QCH + off:hl * QCH + off + P],
                        in0=s01[:, hl * QCH + off:hl * QCH + off + P],
                        in1=tri_sb)
                for hl in (0, 1):
                    a_t = attnp.tile([P, width], MMDT, tag=f"at{ki}",
                                     name=f"attn{t}_{hl}_{ki}")
                    attn[(hl, ki)] = a_t
                    nc.scalar.activation(
                        out=a_t, in_=s01[:, hl * QCH + off:(hl + 1) * QCH],
                        func=AF.Exp, accum_out=rsp[:, hl:hl + 1])
            # next pair's projection slice: PE work while ACT drains exps
            emit_proj_slice(ki)
            # recip+vsc for the PREVIOUS ki (keeps the in-order vector
            # queue from gating the next exp)
            if ki >= 1:
                emit_ri_vsc(ki - 1)
            if ki >= 3:
                emit_z_contrib(ki - 3)
        finish_z_chunk(0)
        if t == NPAIR - 1:
            for st in range(4):
                emit_out_tile(st)
            emit_ri_vsc(NS - 1)
            emit_z_contrib(NS - 3)
            emit_z_contrib(NS - 2)
            emit_z_contrib(NS - 1, last1=True)
            finish_z_chunk(1)
            for st in range(4, NS):
                emit_out_tile(st)
        else:
            emit_ri_vsc(NS - 1)
            emit_z_contrib(NS - 3)
            emit_z_contrib(NS - 2)
            emit_z_contrib(NS - 1, last1=True)
            finish_z_chunk(1)


def prep_inputs(x, W_qkv, b_qkv):
    """Host-side re-layout of inputs (weights de-interleave/transpose/tile)."""
    x = np.ascontiguousarray(np.asarray(x, np.float32)).astype(NP_PDT)
    W = np.asarray(W_qkv, np.float32).reshape(H, DH, 3, DM)
    Wq = W[:, :, 0, :].reshape(H * DH, DM)
    Wk = W[:, :, 1, :].reshape(H * DH, DM)
    Wv = W[:, :, 2, :].reshape(H * DH, DM)
    WqkT = np.ascontiguousarray(np.concatenate([Wq, Wk], 0).T)   # [DM, 2048]
    wqk_host = np.ascontiguousarray(
        WqkT.reshape(ND, P, 2 * NPAIR, P).transpose(2, 1, 0, 3)).astype(NP_PDT)
    wv_host = np.ascontiguousarray(Wv.T).reshape(ND, P, DM).astype(NP_PDT)
    ident = np.eye(P, dtype=np.float32).astype(NP_PDT)
    idx = np.arange(P)
    trimask = np.where(idx[None, :] >= idx[:, None], 0.0, NEG).astype(np.float32)

    b = np.asarray(b_qkv, np.float32).reshape(H, DH, 3)
    bq = b[:, :, 0].reshape(H * DH)
    bk = b[:, :, 1].reshape(H * DH)
    bvv = b[:, :, 2].reshape(H * DH)
    bqk_host = np.ascontiguousarray(
        np.concatenate([bq, bk]).reshape(2 * NPAIR, P).T)         # [P, 16]
    return x, wqk_host, wv_host, ident, trimask, bqk_host, bvv


import ml_dtypes

NP_PDT = ml_dtypes.bfloat16

_prog_cache = {}


def kernel(x, W_qkv, b_qkv, W_o, b_o, trace=False):
    x, wqk_host, wv_host, ident, trimask, bqk_host, bvv = prep_inputs(
        x, W_qkv, b_qkv)
    wo_host = np.ascontiguousarray(np.asarray(W_o, np.float32).T).reshape(ND, P, DM).astype(NP_PDT)
    with_bias = bool(np.any(np.asarray(b_qkv)))
    key = with_bias
    if key not in _prog_cache:
        _prog_cache[key] = build_program(with_bias=with_bias)
    nc = _prog_cache[key]

    in_maps = []
    for bi in range(B):
        m = {
            "xb": x[bi], "wqk": wqk_host, "wv": wv_host, "wo": wo_host,
            "ident": ident, "trimask": trimask,
        }
        if with_bias:
            m["bqk"] = bqk_host
            m["bv"] = bvv.reshape(1, DM).astype(NP_PDT)
        in_maps.append(m)

    res = run_bass_kernel_spmd(nc, in_maps, core_ids=list(range(B)), trace=trace)
    out = np.stack([res.results[bi]["out"] for bi in range(B)]).astype(np.float32)
    out += np.asarray(b_o, np.float32)[None, None, :]
    if trace:
        kernel.last_results = res
    return out


# revision 27
# speedup vs baseline: 1.2377x; 1.0136x over previous
"""Trainium2 Bass kernel for nn_AttentionBlock (RMSNorm + QKV + causal
attention with softmax over the QUERY axis + output projection).

Sharding: data-parallel over batch. B=8 -> one batch element per NeuronCore,
no collectives. Weights are re-laid-out on the host (de-interleave the
(h, dh, 3) QKV packing, transpose so the contraction dim d lands on SBUF
partitions, and pre-tile so every DMA is contiguous).

Device dataflow per core (S=1024, D=1024, H=16, Dh=64):
  1. x natural [s,d] tiles (bf16) -> sum(x^2) via DVE multiply+accum ->
     rsqrt scale -> xbf (bf16) -> PE-transpose (batched 4 per PSUM bank)
     -> normT [d, s] in one unified SBUF tile.
  2. qkT[f, s] = Wqk^T.T @ normT (a 128-row f-tile holds a HEAD PAIR);
     V[s, f] = normT.T @ Wv^T.  Projection loops run the weight tile
     (stationary operand) in the OUTER loop over both 512-wide chunks so
     each LDWEIGHTS serves two matmuls; each projection tile drains its
     2-bank PSUM tile with a single DVE cast.
  3. scores per (pair, k-tile) land in 2-bank PSUM tiles so the full
     valid q-row [ki*128, 1024) is contiguous; causal masking is a DVE
     add of a -1e30 triangle on the diagonal block; ONE Exp per
     (head, k-tile) with accum_out giving the full row sum for free.
  4. normalization folded into V (vsc = V * 1/rowsum);
     z^T[dh, q] = vsc.T @ attnT, two heads into one PSUM bank via
     column-group tiling.
  5. out[s, :] = z.T @ Wo^T accumulated over f tiles, DMA out per chunk.

All matmul operands are bfloat16.  The pair loop is software-pipelined:
the next pair's QKV projections are emitted between the scores and the
final z chunk so the PE always has independent work while ACT drains exps.
"""

import numpy as np
from contextlib import ExitStack

import concourse.bacc as bacc
import concourse.bass as bass
import concourse.tile as tile
from concourse import mybir
from concourse.bass_utils import run_bass_kernel_spmd

B, S, DM, H, DH = 8, 1024, 1024, 16, 64
P = 128
EPS = 1.1920929e-07
NEG = -1e30
F32 = mybir.dt.float32
BF16 = mybir.dt.bfloat16
NS = S // P      # 8 s-tiles (also k-tiles)
ND = DM // P     # 8 d-chunks
NPAIR = H // 2   # 8 head pairs
QCH = 512        # one PSUM bank of fp32
NQC = S // QCH   # 2 q chunks

MMDT = BF16
PDT = BF16


def build_program(with_bias=False):
    nc = bacc.Bacc("TRN2", target_bir_lowering=False, debug=False)

    xb = nc.dram_tensor("xb", [S, DM], PDT, kind="ExternalInput").ap()
    # (ft, dd, dk, f): ft 0-7 = Q pair tiles, 8-15 = K pair tiles
    wqk = nc.dram_tensor("wqk", [2 * NPAIR, P, ND, P], PDT, kind="ExternalInput").ap()
    wv = nc.dram_tensor("wv", [ND, P, DM], PDT, kind="ExternalInput").ap()
    wo = nc.dram_tensor("wo", [ND, P, DM], PDT, kind="ExternalInput").ap()
    ident = nc.dram_tensor("ident", [P, P], PDT, kind="ExternalInput").ap()
    trimask = nc.dram_tensor("trimask", [P, P], F32, kind="ExternalInput").ap()
    bqk = bv = None
    if with_bias:
        bqk = nc.dram_tensor("bqk", [P, 2 * NPAIR], F32, kind="ExternalInput").ap()
        bv = nc.dram_tensor("bv", [1, DM], PDT, kind="ExternalInput").ap()
    out = nc.dram_tensor("out", [S, DM], F32, kind="ExternalOutput").ap()

    with tile.TileContext(nc) as tc:
        with ExitStack() as ctx:
            _build_body(ctx, tc, xb, wqk, wv, wo, ident, trimask, bqk, bv, out)
    nc.compile()
    return nc


def _build_body(ctx, tc, xb, wqk, wv, wo, ident, trimask, bqk, bv, out):
    nc = tc.nc
    AF = mybir.ActivationFunctionType
    ALU = mybir.AluOpType

    singles = ctx.enter_context(tc.tile_pool(name="singles", bufs=1))
    big8 = ctx.enter_context(tc.tile_pool(name="big8", bufs=1))    # V tiles
    xqp = ctx.enter_context(tc.tile_pool(name="xqp", bufs=1))      # x natural
    xbfp = ctx.enter_context(tc.tile_pool(name="xbfp", bufs=2))    # scaled bf16 x
    scr = ctx.enter_context(tc.tile_pool(name="scr", bufs=2))      # out tiles
    nt = ctx.enter_context(tc.tile_pool(name="nt", bufs=1))        # normT
    w8 = ctx.enter_context(tc.tile_pool(name="w8", bufs=1))        # wv then wo
    wqks = ctx.enter_context(tc.tile_pool(name="wqks", bufs=1))    # wqk stream
    qkp = ctx.enter_context(tc.tile_pool(name="qkp", bufs=3))      # qt/kt tiles
    attnp = ctx.enter_context(tc.tile_pool(name="attnp", bufs=2))  # exp(scores)
    ztp = ctx.enter_context(tc.tile_pool(name="ztp", bufs=1))      # z transposed
    vscp = ctx.enter_context(tc.tile_pool(name="vscp", bufs=2))    # scaled V
    sm = ctx.enter_context(tc.tile_pool(name="sm", bufs=3))        # small stats
    ps = ctx.enter_context(tc.tile_pool(name="ps", bufs=1, space="PSUM"))

    # PSUM budget (8 banks): tag "sc" = [P, 1024] f32 (2 banks) x bufs 2
    # (scores only), tag "mm" = [P, 512] x bufs 2 (projection chunks),
    # tag "z" = [P, 512] x bufs 2 (z accum, transposes, last out tile).
    def sc_tile(name):
        return ps.tile([P, 2 * QCH], F32, tag="sc", bufs=2, name=name)

    def mm_bank(name):
        return ps.tile([P, QCH], F32, tag="mm", bufs=2, name=name)

    def z_bank(name):
        return ps.tile([P, QCH], F32, tag="z", bufs=2, name=name)

    # ---------------- Phase A: RMSNorm + transpose ----------------
    # x tile 0 heads the RMS->transpose critical path, so its DMA goes
    # first; ident (needed by the first transpose) right after.
    x_qs = []
    for qi in range(NS // 2):
        x_q = xqp.tile([P, 2, DM], PDT, tag=f"xq{qi}", name=f"xq{qi}")
        x_qs.append(x_q)
    nc.sync.dma_start(out=x_qs[0][:, 0, :], in_=xb[0:P, :])
    ident_sb = singles.tile([P, P], PDT, tag="ident")
    nc.sync.dma_start(out=ident_sb, in_=ident)
    nc.sync.dma_start(out=x_qs[0][:, 1, :], in_=xb[P:2 * P, :])
    for qi in range(1, NS // 2):
        nc.sync.dma_start(
            out=x_qs[qi],
            in_=xb[qi * 2 * P:(qi + 1) * 2 * P, :].rearrange(
                "(j p) d -> p j d", j=2))
    eps_sb = singles.tile([P, 1], F32, tag="eps")
    nc.vector.memset(eps_sb, EPS)
    # touch Sqrt now so its ACT table load (~1.3us) happens during the
    # input DMA.  Exp is warmed AFTER the last phase-A sqrt: the table
    # memory holds one function, so warming it earlier would evict Sqrt
    # and force a reload right on the rs critical path.
    warm_sb = singles.tile([P, 1], F32, tag="warm")
    nc.scalar.activation(out=warm_sb, in_=eps_sb, func=AF.Sqrt)
    tri_sb = singles.tile([P, P], F32, tag="tri")
    nc.sync.dma_start(out=tri_sb, in_=trimask)
    wqk_pre = {}
    for ft in (0, NPAIR, 1, NPAIR + 1):
        w_t = wqks.tile([P, ND, P], PDT, tag="wqk", bufs=4, name=f"wqk{ft}")
        nc.sync.dma_start(out=w_t, in_=wqk[ft])
        wqk_pre[ft] = w_t
    bqk_sb = bv_sb = ones_sb = None
    if bqk is not None:
        bqk_sb = singles.tile([P, 2 * NPAIR], F32, tag="bqk")
        nc.sync.dma_start(out=bqk_sb, in_=bqk)
        bv_sb = singles.tile([1, DM], PDT, tag="bv")
        nc.sync.dma_start(out=bv_sb, in_=bv)
        ones_sb = singles.tile([1, P], PDT, tag="ones")
        nc.vector.memset(ones_sb, 1.0)

    # unified normT: nt_all[:, dk, s]
    nt_all = nt.tile([P, ND, S], PDT, tag="nt", name="normT")


    # ---------------- QK projection helpers ----------------
    # A (which, chunk) group is 8 accumulating matmuls into one mm bank,
    # cast to its half of the qkT destination tile.  Groups are emitted
    # either whole (phase A/B) or sliced 2-dk at a time into the pair
    # loop so the PE has work while ACT drains exps.
    qt_tiles = {}
    kt_tiles = {}

    def dma_wqk(t):
        for ft in (t, NPAIR + t):
            if ft not in wqk_pre:
                w_t = wqks.tile([P, ND, P], PDT, tag="wqk", bufs=4,
                                name=f"wqk{ft}")
                nc.sync.dma_start(out=w_t, in_=wqk[ft])
                wqk_pre[ft] = w_t

    def qk_alloc(t, which):
        tiles = qt_tiles if which == "qt" else kt_tiles
        ft = t if which == "qt" else NPAIR + t
        w_t = wqk_pre.pop(ft)
        dst = qkp.tile([P, S], PDT, tag=which, name=f"{which}{t}")
        tiles[t] = (dst, w_t)
        return dst, w_t

    def qk_mms(t, which, sc, mm, dks):
        w_t = (qt_tiles if which == "qt" else kt_tiles)[t][1]
        for dk in dks:
            nc.tensor.matmul(
                mm,
                w_t[:, dk, :],
                nt_all[:, dk, sc * QCH:(sc + 1) * QCH],
                start=(dk == 0), stop=(dk == ND - 1),
            )

    def qk_cast(t, which, sc, mm):
        dst = (qt_tiles if which == "qt" else kt_tiles)[t][0]
        ft = t if which == "qt" else NPAIR + t
        dv = dst[:, sc * QCH:(sc + 1) * QCH]
        if bqk_sb is not None:
            nc.vector.tensor_scalar_add(out=dv, in0=mm,
                                        scalar1=bqk_sb[:, ft:ft + 1])
        else:
            nc.vector.tensor_copy(out=dv, in_=mm)

    def emit_qk_group(t, which, sc):
        if sc == 0 and t not in (qt_tiles if which == "qt" else kt_tiles):
            qk_alloc(t, which)
        mm = mm_bank(f"qk{which}{t}_{sc}")
        qk_mms(t, which, sc, mm, range(ND))
        qk_cast(t, which, sc, mm)

    for st in range(NS):
        x_t = x_qs[st // 2][:, st % 2, :]
        sqscr = sm.tile([P, DM], PDT, tag="sqscr", bufs=1, name=f"sqscr{st}")
        ssum = sm.tile([P, 1], F32, tag="ssA", name=f"ssA{st}")
        nc.vector.scalar_tensor_tensor(
            out=sqscr, in0=x_t, scalar=1.0, in1=x_t,
            op0=ALU.mult, op1=ALU.mult, accum_out=ssum)
        rs_t = sm.tile([P, 1], F32, tag="rs", name=f"rs{st}")
        # rs = sqrt(mean(x^2) + eps) then reciprocal
        nc.scalar.activation(out=rs_t, in_=ssum, func=AF.Sqrt,
                             bias=eps_sb, scale=1.0 / DM)
        nc.vector.reciprocal(out=rs_t, in_=rs_t)
        xbf = xbfp.tile([P, DM], PDT, tag="xbf", name=f"xbf{st}")
        nc.vector.tensor_scalar_mul(out=xbf, in0=x_t, scalar1=rs_t)
        # batched PE transposes: 4 per PSUM bank, one wide cast to normT
        for g in range(2):
            tp_ps = ps.tile([P, 4, P], PDT, tag="z", bufs=2, name=f"tp{st}_{g}")
            for j in range(4):
                dk = 4 * g + j
                nc.tensor.transpose(tp_ps[:, j, 0:P],
                                    xbf[:, dk * P:(dk + 1) * P], ident_sb)
            nc.scalar.copy(
                out=nt_all[:, 4 * g:4 * g + 4, st * P:(st + 1) * P],
                in_=tp_ps[:, :, 0:P])
        # normT cols 0..511 complete: give the PE pair-0 projection work so
        # it isn't stuck behind transposes that wait on the scalar RMS chain
        if st == 3:
            emit_qk_group(0, "qt", 0)
        if st == 5:
            emit_qk_group(0, "kt", 0)
        if st == NS - 1:
            # all phase-A sqrts emitted: load the Exp table during phase B
            nc.scalar.activation(out=warm_sb, in_=eps_sb, func=AF.Exp)

    # ---------------- Phase B: rest of QK pair 0, then V ----------------
    # (pair 1's projection is interleaved into pair 0's ki loop)
    emit_qk_group(0, "qt", 1)
    emit_qk_group(0, "kt", 1)

    wv_sb = []
    for dk in range(ND):
        w_t = w8.tile([P, DM], PDT, tag=f"w{dk}", name=f"wv{dk}")
        nc.sync.dma_start(out=w_t, in_=wv[dk])
        wv_sb.append(w_t)
    vs = []
    for st in range(NS):
        v_t = big8.tile([P, DM], PDT, tag=f"b{st}", name=f"vs{st}")
        vs.append(v_t)
        mm = sc_tile(f"vps{st}")
        for dk in range(ND):
            for fvc in range(NQC):
                nc.tensor.matmul(
                    mm[:, fvc * QCH:(fvc + 1) * QCH],
                    nt_all[:, dk, st * P:(st + 1) * P],
                    wv_sb[dk][:, fvc * QCH:(fvc + 1) * QCH],
                    start=(dk == 0),
                    stop=(dk == ND - 1 and bv_sb is None),
                )
        if bv_sb is not None:
            for fvc in range(NQC):
                nc.tensor.matmul(
                    mm[:, fvc * QCH:(fvc + 1) * QCH], ones_sb,
                    bv_sb[:, fvc * QCH:(fvc + 1) * QCH],
                    start=False, stop=True,
                )
        nc.vector.tensor_copy(
            out=v_t.rearrange("p (c q) -> p c q", c=2),
            in_=mm.rearrange("p (c q) -> p c q", c=2))

    # prefetch Wo early: the w8 slots free up as soon as V consumed wv
    wo_sb = []
    for fk in range(ND):
        w_t = w8.tile([P, DM], PDT, tag=f"w{fk}", name=f"wo{fk}")
        nc.sync.dma_start(out=w_t, in_=wo[fk])
        wo_sb.append(w_t)

    # ---------------- Phase E emitter (out projection per s-tile) --------
    zT = []
    out_partial = {}

    def emit_out_partial_slice(ki):
        """fk-steps of out tile 0 over pairs 0-6's zT, interleaved into
        the (otherwise projection-free) last pair's ki loop."""
        if ki == 0:
            out_partial[0] = mm_bank("ops0_0")
            out_partial[1] = mm_bank("ops0_1")
        fks = {0: (0, 1), 1: (2, 3), 2: (4, 5), 3: (6,)}.get(ki)
        if fks is None:
            return
        for fk in fks:
            for dmc in range(NQC):
                nc.tensor.matmul(
                    out_partial[dmc],
                    zT[fk][:, 0:P],
                    wo_sb[fk][:, dmc * QCH:(dmc + 1) * QCH],
                    start=(fk == 0), stop=False,
                )

    def emit_out_tile(st):
        o_t = scr.tile([P, DM], F32, tag="osb", name=f"osb{st}")
        if st == 0:
            # close the partial accumulated during the last pair's ki loop
            for dmc in range(NQC):
                nc.tensor.matmul(
                    out_partial[dmc],
                    zT[ND - 1][:, 0:P],
                    wo_sb[ND - 1][:, dmc * QCH:(dmc + 1) * QCH],
                    start=False, stop=True,
                )
            for dmc in range(NQC):
                nc.vector.tensor_copy(out=o_t[:, dmc * QCH:(dmc + 1) * QCH],
                                      in_=out_partial[dmc])
                nc.sync.dma_start(
                    out=out[0:P, dmc * QCH:(dmc + 1) * QCH],
                    in_=o_t[:, dmc * QCH:(dmc + 1) * QCH])
            return
        if st == NS - 1:
            # quarter-width groups on z banks: the copy+DMA chain overlaps
            # the remaining matmuls instead of serializing at kernel end
            w = QCH // 2
            for half in range(2):
                mm = z_bank(f"ops{st}_{half}")
                for sub in range(2):
                    dmc = 2 * half + sub
                    for fk in range(ND):
                        nc.tensor.matmul(
                            mm[:, sub * w:(sub + 1) * w],
                            zT[fk][:, st * P:(st + 1) * P],
                            wo_sb[fk][:, dmc * w:(dmc + 1) * w],
                            start=(fk == 0), stop=(fk == ND - 1),
                        )
                    nc.vector.tensor_copy(
                        out=o_t[:, dmc * w:(dmc + 1) * w],
                        in_=mm[:, sub * w:(sub + 1) * w])
                    nc.sync.dma_start(
                        out=out[st * P:(st + 1) * P, dmc * w:(dmc + 1) * w],
                        in_=o_t[:, dmc * w:(dmc + 1) * w])
            return
        if st in (3, 6):
            # rotate through the mm banks so consecutive out tiles never
            # wait on each other's PSUM drain
            mms = [mm_bank(f"ops{st}_{dmc}") for dmc in range(NQC)]
            for fk in range(ND):
                for dmc in range(NQC):
                    nc.tensor.matmul(
                        mms[dmc],
                        zT[fk][:, st * P:(st + 1) * P],
                        wo_sb[fk][:, dmc * QCH:(dmc + 1) * QCH],
                        start=(fk == 0), stop=(fk == ND - 1),
                    )
            for dmc in range(NQC):
                nc.vector.tensor_copy(out=o_t[:, dmc * QCH:(dmc + 1) * QCH],
                                      in_=mms[dmc])
                nc.sync.dma_start(
                    out=out[st * P:(st + 1) * P, dmc * QCH:(dmc + 1) * QCH],
                    in_=o_t[:, dmc * QCH:(dmc + 1) * QCH])
            return
        mm = sc_tile(f"ops{st}")
        for fk in range(ND):
            for dmc in range(NQC):
                nc.tensor.matmul(
                    mm[:, dmc * QCH:(dmc + 1) * QCH],
                    zT[fk][:, st * P:(st + 1) * P],
                    wo_sb[fk][:, dmc * QCH:(dmc + 1) * QCH],
                    start=(fk == 0), stop=(fk == ND - 1),
                )
        for dmc in range(NQC):
            nc.vector.tensor_copy(out=o_t[:, dmc * QCH:(dmc + 1) * QCH],
                                  in_=mm[:, dmc * QCH:(dmc + 1) * QCH])
            nc.sync.dma_start(
                out=out[st * P:(st + 1) * P, dmc * QCH:(dmc + 1) * QCH],
                in_=o_t[:, dmc * QCH:(dmc + 1) * QCH])

    # ---------------- Phase C/D: attention per head pair ----------------
    # dk-step schedule for the lag-1 interleaved projection: Q over kis
    # 0-2 (cast at 2), K over kis 3-5 (cast at 5) so the next pair's
    # scores never wait on a cast at the pair boundary.
    PROJ_SCHED = {0: ("qt", (0, 1, 2)), 1: ("qt", (3, 4, 5)),
                  2: ("qt", (6, 7)), 3: ("kt", (0, 1, 2)),
                  4: ("kt", (3, 4, 5)), 5: ("kt", (6, 7))}

    for t in range(NPAIR):
        have_proj = t + 1 < NPAIR
        if t + 2 < NPAIR:
            dma_wqk(t + 2)
        if have_proj:
            qk_alloc(t + 1, "qt")
            qk_alloc(t + 1, "kt")
            proj_mm = {}
        qt, kt = qt_tiles.pop(t)[0], kt_tiles.pop(t)[0]

        def emit_proj_slice(ki, t=t):
            """dk-steps of the t+1 projection, interleaved into the ki
            loop.  One LDWEIGHTS per dk serves both 512-wide chunks."""
            if not have_proj or ki not in PROJ_SCHED:
                return
            which, dks = PROJ_SCHED[ki]
            if dks[0] == 0:
                proj_mm[(which, 0)] = mm_bank(f"qk{which}{t + 1}_0")
                proj_mm[(which, 1)] = mm_bank(f"qk{which}{t + 1}_1")
            w_t = (qt_tiles if which == "qt" else kt_tiles)[t + 1][1]
            for dk in dks:
                for sc in range(NQC):
                    nc.tensor.matmul(
                        proj_mm[(which, sc)],
                        w_t[:, dk, :],
                        nt_all[:, dk, sc * QCH:(sc + 1) * QCH],
                        start=(dk == 0), stop=(dk == ND - 1),
                    )
            if dks[-1] == ND - 1:
                for sc in range(NQC):
                    qk_cast(t + 1, which, sc, proj_mm[(which, sc)])

        z_t = ztp.tile([P, S], PDT, tag=f"zt{t}", name=f"zT{t}")
        zT.append(z_t)
        attn = {}   # (head_local, ki) -> sbuf tile [P, width]
        vsc_d = {}  # ki -> scaled V slice [P, 128] for this pair
        rsp_d = {}  # ki -> row sums [P, 2] (hl 0, 1)

        def emit_ri_vsc(ki, t=t, vsc_d=vsc_d, rsp_d=rsp_d):
            ri = sm.tile([P, 2], F32, tag="ri", name=f"ri{t}_{ki}")
            nc.vector.reciprocal(out=ri, in_=rsp_d[ki])
            vsc = vscp.tile([P, P], MMDT, tag=f"vsc{ki}", name=f"vsc{t}_{ki}")
            vsc_d[ki] = vsc
            ri_b = bass.AP(tensor=ri.tensor, offset=ri.offset,
                           ap=[list(ri.ap[0]), list(ri.ap[1]), [0, DH]])
            nc.vector.tensor_tensor(
                out=vsc.rearrange("p (h d) -> p h d", h=2),
                in0=vs[ki][:, t * P:(t + 1) * P].rearrange(
                    "p (h d) -> p h d", h=2),
                in1=ri_b, op=ALU.mult)

        z_ps = {}
        for qc in range(NQC):
            z_ps[qc] = z_bank(f"zps{t}_{qc}")

        def emit_z_contrib(ki, last1=False, t=t, attn=attn, vsc_d=vsc_d,
                           z_ps=z_ps):
            chunks = ([(0, ki == 3)] if ki < 4 else []) + [(1, last1)]
            for qc, stop in chunks:
                q0 = max(qc * QCH, ki * P)
                for hl in (0, 1):
                    nc.tensor.matmul(
                        z_ps[qc][hl * DH:(hl + 1) * DH, q0 - qc * QCH:QCH],
                        vsc_d[ki][:, hl * DH:(hl + 1) * DH],
                        attn[(hl, ki)][:, q0 - ki * P:(qc + 1) * QCH - ki * P],
                        start=(ki == 0), stop=stop,
                    )

        def finish_z_chunk(qc, t=t, z_t=z_t, z_ps=z_ps):
            # chunk 0's cast goes to scalar (emitted after the pair's last
            # exp, so it can't delay them); chunk 1's to vector
            if qc == 0 and t != NPAIR - 1:
                nc.scalar.copy(out=z_t[:, qc * QCH:(qc + 1) * QCH],
                               in_=z_ps[qc])
            else:
                nc.vector.tensor_copy(
                    out=z_t[:, qc * QCH:(qc + 1) * QCH], in_=z_ps[qc])

        for ki in range(NS):
            width = S - ki * P
            rsp = sm.tile([P, 2], F32, tag="rsp", name=f"rsp{t}_{ki}")
            rsp_d[ki] = rsp
            if ki < 4:
                # two 2-bank tiles, one per head; each holds the full
                # valid q span [ki*128, 1024) contiguously
                s_hl = [sc_tile(f"sps{t}_{hl}_{ki}") for hl in (0, 1)]
                # hl outer: one LDWEIGHTS of the kt slice serves both qc
                for hl, prange in ((0, slice(0, DH)), (1, slice(DH, P))):
                    for qc in range(NQC):
                        q0 = max(qc * QCH, ki * P)
                        nc.tensor.matmul(
                            s_hl[hl][:, q0:(qc + 1) * QCH],
                            kt[prange, ki * P:(ki + 1) * P],
                            qt[prange, q0:(qc + 1) * QCH],
                            start=True, stop=True,
                        )
                for hl in (0, 1):
                    nc.vector.tensor_add(
                        out=s_hl[hl][:, ki * P:(ki + 1) * P],
                        in0=s_hl[hl][:, ki * P:(ki + 1) * P], in1=tri_sb)
                for hl in (0, 1):
                    a_t = attnp.tile([P, width], MMDT, tag=f"at{ki}",
                                     name=f"attn{t}_{hl}_{ki}")
                    attn[(hl, ki)] = a_t
                    nc.scalar.activation(
                        out=a_t, in_=s_hl[hl][:, ki * P:2 * QCH],
                        func=AF.Exp, accum_out=rsp[:, hl:hl + 1])
            else:
                # one 2-bank tile: bank 0 = head 0, bank 1 = head 1
                s01 = sc_tile(f"sps{t}_{ki}")
                off = ki * P - QCH
                for hl, prange in ((0, slice(0, DH)), (1, slice(DH, P))):
                    nc.tensor.matmul(
                        s01[:, hl * QCH + off:(hl + 1) * QCH],
                        kt[prange, ki * P:(ki + 1) * P],
                        qt[prange, ki * P:S],
                        start=True, stop=True,
                    )
                for hl in (0, 1):
                    nc.vector.tensor_add(
                        out=s01[:, hl * QCH + off:hl * QCH + off + P],
                        in0=s01[:, hl * QCH + off:hl * QCH + off + P],
                        in1=tri_sb)
                for hl in (0, 1):
                    a_t = attnp.tile([P, width], MMDT, tag=f"at{ki}",
                                     name=f"attn{t}_{hl}_{ki}")
                    attn[(hl, ki)] = a_t
                    nc.scalar.activation(
                        out=a_t, in_=s01[:, hl * QCH + off:(hl + 1) * QCH],
                        func=AF.Exp, accum_out=rsp[:, hl:hl + 1])
            # next pair's projection slice: PE work while ACT drains exps
            emit_proj_slice(ki)
            # the last pair has no projection: pre-accumulate out tile 0
            if t == NPAIR - 1:
                emit_out_partial_slice(ki)
            # recip+vsc for the PREVIOUS ki (keeps the in-order vector
            # queue from gating the next exp)
            if ki >= 1:
                emit_ri_vsc(ki - 1)
            if ki >= 3:
                emit_z_contrib(ki - 3)
        finish_z_chunk(0)
        if t == NPAIR - 1:
            for st in range(4):
                emit_out_tile(st)
            emit_ri_vsc(NS - 1)
            emit_z_contrib(NS - 3)
            emit_z_contrib(NS - 2)
            emit_z_contrib(NS - 1, last1=True)
            finish_z_chunk(1)
            for st in range(4, NS):
                emit_out_tile(st)
        else:
            emit_ri_vsc(NS - 1)
            emit_z_contrib(NS - 3)
            emit_z_contrib(NS - 2)
            emit_z_contrib(NS - 1, last1=True)
            finish_z_chunk(1)


def prep_inputs(x, W_qkv, b_qkv):
    """Host-side re-layout of inputs (weights de-interleave/transpose/tile)."""
    x = np.ascontiguousarray(np.asarray(x, np.float32)).astype(NP_PDT)
    W = np.asarray(W_qkv, np.float32).reshape(H, DH, 3, DM)
    Wq = W[:, :, 0, :].reshape(H * DH, DM)
    Wk = W[:, :, 1, :].reshape(H * DH, DM)
    Wv = W[:, :, 2, :].reshape(H * DH, DM)
    WqkT = np.ascontiguousarray(np.concatenate([Wq, Wk], 0).T)   # [DM, 2048]
    wqk_host = np.ascontiguousarray(
        WqkT.reshape(ND, P, 2 * NPAIR, P).transpose(2, 1, 0, 3)).astype(NP_PDT)
    wv_host = np.ascontiguousarray(Wv.T).reshape(ND, P, DM).astype(NP_PDT)
    ident = np.eye(P, dtype=np.float32).astype(NP_PDT)
    idx = np.arange(P)
    trimask = np.where(idx[None, :] >= idx[:, None], 0.0, NEG).astype(np.float32)

    b = np.asarray(b_qkv, np.float32).reshape(H, DH, 3)
    bq = b[:, :, 0].reshape(H * DH)
    bk = b[:, :, 1].reshape(H * DH)
    bvv = b[:, :, 2].reshape(H * DH)
    bqk_host = np.ascontiguousarray(
        np.concatenate([bq, bk]).reshape(2 * NPAIR, P).T)         # [P, 16]
    return x, wqk_host, wv_host, ident, trimask, bqk_host, bvv


import ml_dtypes

NP_PDT = ml_dtypes.bfloat16

_prog_cache = {}


def kernel(x, W_qkv, b_qkv, W_o, b_o, trace=False):
    x, wqk_host, wv_host, ident, trimask, bqk_host, bvv = prep_inputs(
        x, W_qkv, b_qkv)
    wo_host = np.ascontiguousarray(np.asarray(W_o, np.float32).T).reshape(ND, P, DM).astype(NP_PDT)
    with_bias = bool(np.any(np.asarray(b_qkv)))
    key = with_bias
    if key not in _prog_cache:
        _prog_cache[key] = build_program(with_bias=with_bias)
    nc = _prog_cache[key]

    in_maps = []
    for bi in range(B):
        m = {
            "xb": x[bi], "wqk": wqk_host, "wv": wv_host, "wo": wo_host,
            "ident": ident, "trimask": trimask,
        }
        if with_bias:
            m["bqk"] = bqk_host
            m["bv"] = bvv.reshape(1, DM).astype(NP_PDT)
        in_maps.append(m)

    res = run_bass_kernel_spmd(nc, in_maps, core_ids=list(range(B)), trace=trace)
    out = np.stack([res.results[bi]["out"] for bi in range(B)]).astype(np.float32)
    out += np.asarray(b_o, np.float32)[None, None, :]
    if trace:
        kernel.last_results = res
    return out
